# revision 22
# baseline (speedup 1.0000x reference)
"""Trainium2 Bass kernel for nn_Block_27848567948000 (dense transformer block).

Sharding (8 NeuronCores): 4 data-parallel groups over batch (B=4), 2-way
tensor-parallel within each pair: attention sharded over heads (5 each).
out_proj computed as per-head partial sums over ALL T, summed + token-scattered
via a pairwise ReduceScatter; MLP over the core's T/2 token half.

kernel(**inputs) takes FULL inputs and returns the FULL (4, 2048, 1280) output.
"""
import sys

sys.path.insert(0, '/opt/trn_rl_repo')

import numpy as np
import ml_dtypes

import concourse.bass as bass
import concourse.tile as tile
from concourse import mybir, bacc
from concourse import bass_utils
from concourse.masks import make_identity

B, T, C, H, D, F = 4, 2048, 1280, 10, 128, 5120
EPS = 1e-5
N_CORES = 8
HPC = H // 2            # heads per core (5)
CPC = HPC * D           # channels per core (640)
f32 = mybir.dt.float32
bf16 = mybir.dt.bfloat16
fp8 = mybir.dt.float8e4
i32 = mybir.dt.int32
AF = mybir.ActivationFunctionType
OP = mybir.AluOpType
AX = mybir.AxisListType

NT = T // 128            # 16 token tiles
NH = T // 2 // 128       # 8 token tiles in my half
QB = T // 512            # 4 query blocks
NBLK = HPC * 4           # 20 mxfp8 blocks per tensor per token
INV_SQRT_D = float(1.0 / np.sqrt(D))
NEG = -30000.0


def _ap(t_ap, offset_delta, pattern):
    return bass.AP(tensor=t_ap.tensor, offset=t_ap.offset + offset_delta,
                   ap=pattern)


def _rsqrt_vec(nc, pool, out_ap, in_ap, scale, eps, tag, eng=None):
    """out = 1/sqrt(in*scale + eps) on a DVE-like engine (no act tables).
    Bit-trick seed + 2 Newton iterations (~1e-6 rel err). Shapes (128, n)."""
    if eng is None:
        eng = nc.vector
    i32_ = mybir.dt.int32
    shp = [128, in_ap.free_size()]
    m = pool.tile(shp, f32, tag=tag + 'm', name='rs_m')
    eng.tensor_scalar(out=m[:], in0=in_ap, scalar1=scale, scalar2=eps,
                      op0=OP.mult, op1=OP.add)
    y = pool.tile(shp, f32, tag=tag + 'y', name='rs_y')
    eng.tensor_single_scalar(out=y[:].bitcast(i32_),
                             in_=m[:].bitcast(i32_), scalar=1,
                             op=OP.logical_shift_right)
    eng.tensor_scalar(out=y[:].bitcast(i32_), in0=y[:].bitcast(i32_),
                      scalar1=-1, scalar2=0x5f3759df,
                      op0=OP.mult, op1=OP.add)
    t = pool.tile(shp, f32, tag=tag + 't', name='rs_t')
    for it in range(2):
        eng.tensor_tensor(out=t[:], in0=y[:], in1=y[:], op=OP.mult)
        eng.tensor_tensor(out=t[:], in0=t[:], in1=m[:], op=OP.mult)
        eng.tensor_scalar(out=t[:], in0=t[:], scalar1=-0.5,
                          scalar2=1.5, op0=OP.mult, op1=OP.add)
        eng.tensor_tensor(out=y[:] if it == 0 else out_ap, in0=y[:],
                          in1=t[:], op=OP.mult)


def build_nc(t_len=T, n_cores=N_CORES):
    import contextlib
    nc = bacc.Bacc('TRN2', target_bir_lowering=False, debug=False,
                   num_devices=n_cores)

    # ---- DRAM I/O ----
    x_d = nc.dram_tensor('x', [T, C], f32, kind='ExternalInput')
    xh_d = nc.dram_tensor('xh', [T // 2, C], f32, kind='ExternalInput')
    wqkv_d = nc.dram_tensor('w_qkv', [C, 3 * CPC], bf16, kind='ExternalInput')
    cosq_d = nc.dram_tensor('cosq', [T, D], bf16, kind='ExternalInput')
    sinq_d = nc.dram_tensor('sinq', [T, D], bf16, kind='ExternalInput')
    cosk_d = nc.dram_tensor('cosk', [T, D], bf16, kind='ExternalInput')
    sink_d = nc.dram_tensor('sink', [T, D], bf16, kind='ExternalInput')
    wout_d = nc.dram_tensor('w_out', [CPC, C], bf16, kind='ExternalInput')
    wfc1_d = nc.dram_tensor('w_fc1', [C, F], bf16, kind='ExternalInput')
    wfc2_d = nc.dram_tensor('w_fc2', [F, C], bf16, kind='ExternalInput')
    y_d = nc.dram_tensor('y', [T // 2, C], f32, kind='ExternalOutput')

    with tile.TileContext(nc) as tc:
        with contextlib.ExitStack() as ctx:
            persist = ctx.enter_context(tc.tile_pool(name='persist', bufs=1))
            dram = ctx.enter_context(tc.tile_pool(name='dram', bufs=1,
                                                  space='DRAM'))

            # ---- constants ----
            ident_b = persist.tile([128, 128], bf16)
            make_identity(nc, ident_b)
            ident_f = persist.tile([128, 128], f32)
            make_identity(nc, ident_f)
            ones128 = persist.tile([128, 128], bf16)
            nc.vector.memset(ones128[:], 1.0)
            zero_sb = persist.tile([128, 1], f32)
            nc.vector.memset(zero_sb[:], 0.0)
            eps_sb = persist.tile([128, 1], f32)
            nc.vector.memset(eps_sb[:], EPS)
            scr_sq = persist.tile([128, C], bf16)   # Square-output scratch

            # DRAM scratch for the collective
            rs_in = dram.tile([T, C], bf16)
            rs_out = dram.tile([T // 2, C], bf16)

            with contextlib.ExitStack() as pab:
                ab = pab.enter_context(tc.tile_pool(name='ab', bufs=1))
                qT = ab.tile([128, HPC, T], bf16)
                kT = ab.tile([128, HPC, T], bf16)
                vd_sb = ab.tile([128, NT, HPC, 130], bf16)
                attnT = ab.tile([128, HPC, T], bf16)
                nc.vector.memset(vd_sb[:, :, :, 128:129], 1.0)

                # ====== phases A+B ======
                with contextlib.ExitStack() as pin:
                    a_w = pin.enter_context(tc.tile_pool(name='a_w', bufs=1))
                    wq_sb = a_w.tile([128, 10, 3 * CPC], bf16)
                    nc.sync.dma_start(
                        out=wq_sb[:],
                        in_=wqkv_d.ap().rearrange('(j p) c -> p j c', p=128))
                    cq_sb = a_w.tile([128, NT, D], bf16)
                    nc.sync.dma_start(
                        out=cq_sb[:],
                        in_=cosq_d.ap().rearrange('(t p) d -> p t d', p=128))
                    sq_sb = a_w.tile([128, NT, D], bf16)
                    nc.sync.dma_start(
                        out=sq_sb[:],
                        in_=sinq_d.ap().rearrange('(t p) d -> p t d', p=128))
                    ck_sb = a_w.tile([128, NT, D], bf16)
                    nc.sync.dma_start(
                        out=ck_sb[:],
                        in_=cosk_d.ap().rearrange('(t p) d -> p t d', p=128))
                    sk_sb = a_w.tile([128, NT, D], bf16)
                    nc.sync.dma_start(
                        out=sk_sb[:],
                        in_=sink_d.ap().rearrange('(t p) d -> p t d', p=128))

                    a_t = pin.enter_context(tc.tile_pool(name='a_t', bufs=2))
                    a_s = pin.enter_context(tc.tile_pool(name='a_s', bufs=2))
                    a_q = pin.enter_context(tc.tile_pool(name='a_q', bufs=2))
                    pT_pool = pin.enter_context(
                        tc.tile_pool(name='pT', bufs=3))
                    b_t = pin.enter_context(tc.tile_pool(name='b_t', bufs=2))
                    ps512 = pin.enter_context(
                        tc.tile_pool(name='ps512', bufs=3, space='PSUM'))
                    ops_ps = pin.enter_context(
                        tc.tile_pool(name='ops_ps', bufs=1, space='PSUM'))
                    psT = pin.enter_context(
                        tc.tile_pool(name='psT', bufs=3, space='PSUM'))
                    psD = pin.enter_context(
                        tc.tile_pool(name='psD', bufs=1, space='PSUM'))

                    stash = {}

                    def emit_head(t):
                        xt = a_s.tile([128, C], f32, tag='xt')
                        nc.sync.dma_start(out=xt[:],
                                          in_=x_d[t * 128:(t + 1) * 128, :])
                        ssq = a_s.tile([128, 1], f32, tag='ssq')
                        nc.scalar.activation(out=scr_sq[:], in_=xt[:],
                                             func=AF.Square, bias=zero_sb[:],
                                             accum_out=ssq[:])
                        rstd = a_s.tile([128, 1], f32, tag='rstd')
                        _rsqrt_vec(nc, a_s, rstd[:], ssq[:],
                                   float(1.0 / C), EPS, 'rx')
                        xnT = a_s.tile([128, 10, 128], bf16, tag='xnT')
                        for jg, (lo, hi) in enumerate(((0, 4), (4, 8),
                                                      (8, 10))):
                            tp = psT.tile([128, 512], f32, tag='tp',
                                          name='tpf')
                            for j in range(lo, hi):
                                nc.tensor.transpose(
                                    tp[:, (j - lo) * 128:(j - lo + 1) * 128],
                                    xt[:, j * 128:(j + 1) * 128], ident_f[:])
                            nc.scalar.copy(
                                out=xnT[:, lo:hi, :],
                                in_=tp[:, 0:(hi - lo) * 128].rearrange(
                                    'p (j d) -> p j d', d=128))
                        # QKV (chunk-outer, j-mid, g-inner: LDW amortized)
                        qf = a_q.tile([128, CPC], bf16, tag='qf')
                        kf = a_q.tile([128, CPC], bf16, tag='kf')
                        vf = a_q.tile([128, CPC], bf16, tag='vf')
                        dsts = (qf, kf, vf)
                        for lo, hi in ((0, 512), (512, 640)):
                            pss = [ps512.tile([128, 512], f32, tag='mm',
                                              name='qkvps')
                                   for _ in range(3)]
                            for j in range(10):
                                for g in range(3):
                                    nc.tensor.matmul(
                                        pss[g][:, 0:hi - lo], xnT[:, j, :],
                                        wq_sb[:, j,
                                              g * CPC + lo:g * CPC + hi],
                                        start=(j == 0), stop=(j == 9))
                            for g in range(3):
                                if g == 2:
                                    nc.vector.tensor_scalar_mul(
                                        out=dsts[g][:, lo:hi],
                                        in0=pss[g][:, 0:hi - lo],
                                        scalar1=rstd[:])
                                else:
                                    nc.scalar.activation(
                                        out=dsts[g][:, lo:hi],
                                        in_=pss[g][:, 0:hi - lo],
                                        func=AF.Copy, scale=rstd[:])
                        stash[t] = (qf, kf, vf)

                    def rope(eng, src, cos_t, sin_t, out):
                        # out[p,h,d] = src*cos + swap(src)*sinneg   (bf16)
                        src3 = src[:].rearrange('p (h d) -> p h d', h=HPC)
                        pa = list(src3.ap)
                        swap = _ap(src3, 64, pa[:2] + [[-64, 2], [1, 64]])
                        ca = list(cos_t.ap)
                        cos4 = _ap(cos_t, 0, [ca[0], [0, HPC], [1, 128]])
                        sin4 = _ap(sin_t, 0,
                                   [ca[0], [0, HPC], [64, 2], [1, 64]])
                        tmp = a_t.tile([128, HPC, D], bf16, tag='rtmp')
                        eng.tensor_tensor(
                            out=tmp[:].rearrange('p h (u d) -> p h u d', u=2),
                            in0=swap, in1=sin4, op=OP.mult)
                        eng.tensor_tensor(out=out[:], in0=src3, in1=cos4,
                                          op=OP.mult)
                        eng.tensor_add(out=out[:], in0=out[:], in1=tmp[:])

                    def blk4(ap20, reps=32):
                        # (128,20) -> (128,5,4,reps) block broadcast
                        a = list(ap20.ap)
                        st = a[-1][0]
                        return bass.AP(tensor=ap20.tensor, offset=ap20.offset,
                                       ap=[a[0], [4 * st, HPC], [st, 4],
                                           [0, reps]])

                    def hb(ap5, reps=4):
                        # (128,5) -> (128,5,reps) broadcast
                        a = list(ap5.ap)
                        return bass.AP(tensor=ap5.tensor, offset=ap5.offset,
                                       ap=[a[0], [a[-1][0], HPC], [0, reps]])

                    def v4(x):
                        return x.rearrange('p h (b e) -> p h b e', e=32)

                    def emit_tail(t):
                        qf, kf, vf = stash.pop(t)
                        # rms of pre-rope q/k (rope is norm-preserving)
                        msq = a_t.tile([128, 2, HPC], f32, tag='msq')
                        for h in range(HPC):
                            nc.scalar.activation(
                                out=scr_sq[:, 0:D],
                                in_=qf[:, h * D:(h + 1) * D],
                                func=AF.Square, bias=zero_sb[:],
                                accum_out=msq[:, 0, h:h + 1])
                        ksq = a_t.tile([128, HPC, D], bf16, tag='ys', name='ksq')
                        kf3 = kf[:].rearrange('p (h d) -> p h d', h=HPC)
                        nc.vector.tensor_tensor(out=ksq[:], in0=kf3,
                                                in1=kf3, op=OP.mult)
                        nc.vector.tensor_reduce(out=msq[:, 1, :],
                                                in_=ksq[:], axis=AX.X,
                                                op=OP.add)
                        _rsqrt_vec(nc, a_t, msq[:], msq[:],
                                   float(1.0 / D), EPS, 'rqk')
                        # rope (q on vector, k on gpsimd)
                        zq = a_t.tile([128, HPC, D], bf16, tag='zq')
                        rope(nc.vector, qf, cq_sb[:, t, :], sq_sb[:, t, :],
                             zq)
                        zk = a_t.tile([128, HPC, D], bf16, tag='zk')
                        rope(nc.gpsimd, kf, ck_sb[:, t, :], sk_sb[:, t, :],
                             zk)
                        # block amax; amn = amax*rstd (q,k) or amax (v)
                        amn = a_t.tile([128, 3, NBLK], f32, tag='amn')
                        nc.vector.tensor_reduce(
                            out=amn[:, 0, :], in_=v4(zq[:]), axis=AX.X,
                            op=OP.max, apply_absolute_value=True)
                        nc.vector.tensor_reduce(
                            out=amn[:, 1, :], in_=v4(zk[:]), axis=AX.X,
                            op=OP.max, apply_absolute_value=True)
                        nc.vector.tensor_reduce(
                            out=amn[:, 2, :],
                            in_=vf[:].rearrange('p (h b e) -> p h b e',
                                                h=HPC, e=32),
                            axis=AX.X, op=OP.max, apply_absolute_value=True)
                        for i in range(2):
                            nc.vector.tensor_tensor(
                                out=amn[:, i, :].rearrange(
                                    'p (h b) -> p h b', h=HPC),
                                in0=amn[:, i, :].rearrange(
                                    'p (h b) -> p h b', h=HPC),
                                in1=hb(msq[:, i, :]), op=OP.mult)
                        nc.vector.tensor_scalar_max(out=amn[:], in0=amn[:],
                                                    scalar1=1e-12)
                        eb = a_t.tile([128, 3, NBLK], i32, tag='eb')
                        nc.vector.tensor_single_scalar(
                            out=eb[:], in_=amn[:].bitcast(i32), scalar=23,
                            op=OP.logical_shift_right)
                        sc = a_t.tile([128, 3, NBLK], f32, tag='sc')
                        nc.vector.tensor_scalar(
                            out=sc[:].bitcast(i32), in0=eb[:], scalar1=-1,
                            scalar2=260, op0=OP.mult, op1=OP.add)
                        nc.vector.tensor_single_scalar(
                            out=sc[:].bitcast(i32), in_=sc[:].bitcast(i32),
                            scalar=23, op=OP.logical_shift_left)
                        isc = a_t.tile([128, 3, NBLK], f32, tag='isc')
                        nc.vector.tensor_single_scalar(
                            out=isc[:].bitcast(i32), in_=eb[:], scalar=6,
                            op=OP.subtract)
                        nc.vector.tensor_single_scalar(
                            out=isc[:].bitcast(i32), in_=isc[:].bitcast(i32),
                            scalar=23, op=OP.logical_shift_left)
                        msc = a_t.tile([128, 2, NBLK], f32, tag='msc')
                        for i in range(2):
                            nc.vector.tensor_tensor(
                                out=msc[:, i, :].rearrange(
                                    'p (h b) -> p h b', h=HPC),
                                in0=sc[:, i, :].rearrange(
                                    'p (h b) -> p h b', h=HPC),
                                in1=hb(msq[:, i, :]), op=OP.mult)
                        # quantize q (vector)
                        ys = a_t.tile([128, HPC, D], bf16, tag='ys')
                        q8 = a_t.tile([128, HPC, D], fp8, tag='q8')
                        qd = a_t.tile([128, HPC, D], bf16, tag='qd')
                        nc.vector.tensor_tensor(out=v4(ys[:]), in0=v4(zq[:]),
                                                in1=blk4(msc[:, 0, :]),
                                                op=OP.mult)
                        nc.vector.tensor_scalar(out=q8[:], in0=ys[:],
                                                scalar1=-112.0,
                                                scalar2=112.0,
                                                op0=OP.max, op1=OP.min)
                        nc.vector.tensor_tensor(out=v4(qd[:]), in0=v4(q8[:]),
                                                in1=blk4(isc[:, 0, :]),
                                                op=OP.mult)
                        # quantize k (gpsimd mults, vector fp8 cast)
                        ysk = a_t.tile([128, HPC, D], bf16, tag='ys')
                        k8 = a_t.tile([128, HPC, D], fp8, tag='q8')
                        kd = a_t.tile([128, HPC, D], bf16, tag='kd')
                        nc.gpsimd.tensor_tensor(out=v4(ysk[:]),
                                                in0=v4(zk[:]),
                                                in1=blk4(msc[:, 1, :]),
                                                op=OP.mult)
                        nc.vector.tensor_scalar(out=k8[:], in0=ysk[:],
                                                scalar1=-112.0,
                                                scalar2=112.0,
                                                op0=OP.max, op1=OP.min)
                        nc.gpsimd.tensor_tensor(out=v4(kd[:]), in0=v4(k8[:]),
                                                in1=blk4(isc[:, 1, :]),
                                                op=OP.mult)
                        # quantize v (vector; deq straight into vd_sb)
                        ysv = a_t.tile([128, HPC, D], bf16, tag='ys')
                        v8 = a_t.tile([128, HPC, D], fp8, tag='q8')
                        nc.vector.tensor_tensor(
                            out=v4(ysv[:]),
                            in0=v4(vf[:].rearrange('p (h d) -> p h d',
                                                   h=HPC)),
                            in1=blk4(sc[:, 2, :]), op=OP.mult)
                        nc.vector.tensor_scalar(out=v8[:], in0=ysv[:],
                                                scalar1=-112.0,
                                                scalar2=112.0,
                                                op0=OP.max, op1=OP.min)
                        nc.vector.tensor_tensor(
                            out=v4(vd_sb[:, t, :, 0:D]), in0=v4(v8[:]),
                            in1=blk4(isc[:, 2, :]), op=OP.mult)
                        # transpose qd/kd into qT/kT
                        for src, dstT in ((qd, qT), (kd, kT)):
                            tp = psT.tile([128, 640], bf16, tag='tp')
                            for h in range(HPC):
                                nc.tensor.transpose(
                                    tp[:, h * 128:(h + 1) * 128],
                                    src[:, h, :], ident_b[:])
                            nc.vector.tensor_copy(
                                out=dstT[:, :, t * 128:(t + 1) * 128],
                                in_=tp[:].rearrange('p (h d) -> p h d',
                                                    h=HPC))

                    def emit_attn_h(qb, h):
                        nkt = 4 * qb + 4
                        if True:
                            dps = psD.tile([128, 512], f32, tag='dps')
                            ops = ops_ps.tile([128, 512], f32, tag='ops')
                            for kt in range(nkt):
                                sp = ps512.tile([128, 512], f32, tag='mm')
                                o = kt - 4 * qb
                                nc.tensor.matmul(
                                    sp[:],
                                    kT[:, h, kt * 128:(kt + 1) * 128],
                                    qT[:, h, qb * 512:(qb + 1) * 512],
                                    start=True, stop=True)
                                pT = pT_pool.tile([128, 512], bf16, tag='pT')
                                nc.scalar.activation(out=pT[:], in_=sp[:],
                                                     func=AF.Exp,
                                                     bias=zero_sb[:],
                                                     scale=INV_SQRT_D)
                                if o >= 0:
                                    if qb == QB - 1:
                                        nc.vector.tensor_tensor(
                                            out=pT[:], in0=pT[:],
                                            in1=mask01[:, o, :],
                                            op=OP.mult)
                                    else:
                                        nc.gpsimd.affine_select(
                                            out=pT[:], in_=pT[:],
                                            compare_op=OP.is_ge, fill=0.0,
                                            base=-128 * o,
                                            pattern=[[1, 512]],
                                            channel_multiplier=-1)
                                nc.tensor.matmul(dps[:], ones128[:], pT[:],
                                                 start=(kt == 0),
                                                 stop=(kt == nkt - 1))
                                nc.tensor.matmul(ops[:],
                                                 vd_sb[:, kt, h, 0:128],
                                                 pT[:],
                                                 start=(kt == 0),
                                                 stop=(kt == nkt - 1))
                            rd = b_t.tile([128, 512], f32, tag='rd')
                            nc.vector.reciprocal_approx_fast(out=rd[:],
                                                             in_=dps[:])
                            nc.vector.tensor_tensor(
                                out=attnT[:, h, qb * 512:(qb + 1) * 512],
                                in0=ops[:], in1=rd[:], op=OP.mult)

                    mask01 = a_w.tile([128, 4, 512], bf16)
                    nc.vector.memset(mask01[:], 1.0)
                    for o in range(4):
                        nc.gpsimd.affine_select(
                            out=mask01[:, o, :], in_=mask01[:, o, :],
                            compare_op=OP.is_ge, fill=0.0, base=-128 * o,
                            pattern=[[1, 512]], channel_multiplier=-1)
                    wo_sb = a_w.tile([128, HPC, C], bf16)
                    nc.sync.dma_start(
                        out=wo_sb[:],
                        in_=wout_d.ap().rearrange('(h p) c -> p h c', p=128))
                    # rs_in row layout: [t0:512 | t1024:1536 | t512:1024
                    # | t1536:2048] so each RS half is contiguous.
                    rowblk = {tt: i for i, tt in enumerate(
                        (0, 1, 2, 3, 8, 9, 10, 11, 4, 5, 6, 7,
                         12, 13, 14, 15))}
                    grp = [[2 * i, 2 * i + 1] for i in range(n_cores // 2)]

                    def oproj(tt):
                        ob = b_t.tile([128, C], bf16, tag='ob', name='ob')
                        for ci, (lo, hi) in enumerate(((0, 512),
                                                       (512, 1024),
                                                       (1024, C))):
                            ps = ps512.tile([128, 512], f32, tag='mm',
                                            name='oprojps')
                            for h in range(HPC):
                                nc.tensor.matmul(
                                    ps[:, 0:hi - lo],
                                    attnT[:, h, tt * 128:(tt + 1) * 128],
                                    wo_sb[:, h, lo:hi],
                                    start=(h == 0), stop=(h == HPC - 1))
                            if ci == 2:
                                nc.scalar.copy(out=ob[:, lo:hi],
                                               in_=ps[:, 0:hi - lo])
                            else:
                                nc.vector.tensor_copy(
                                    out=ob[:, lo:hi], in_=ps[:, 0:hi - lo])
                        r = rowblk[tt]
                        nc.sync.dma_start(
                            out=rs_in[r * 128:(r + 1) * 128, :], in_=ob[:])

                    # ---- interleaved A+B+C emission ----
                    # B/C tasks spread ~2 per A-tile to keep PE dense
                    tasks = []   # ('attn', qb, h) / ('oproj', tt) / ('rsA',)
                    for qb in range(QB):
                        for h in range(HPC):
                            tasks.append(('attn', qb, h, 4 * qb + 3))
                        for tt in range(4 * qb, 4 * qb + 4):
                            tasks.append(('oproj', tt, None, 4 * qb + 3))


                    def run_task(tk):
                        kind = tk[0]
                        if kind == 'attn':
                            emit_attn_h(tk[1], tk[2])
                        else:
                            oproj(tk[1])

                    done_tile = -1
                    for t in range(NT):
                        emit_head(t)
                        if t >= 1:
                            emit_tail(t - 1)
                            done_tile = t - 1
                        navail = sum(1 for tk in tasks
                                     if tk[3] <= done_tile)
                        nslots = NT - t
                        per = max(2, -(-navail // max(nslots, 1)))
                        n = 0
                        while tasks and tasks[0][3] <= done_tile and n < per:
                            run_task(tasks.pop(0))
                            n += 1
                    emit_tail(NT - 1)
                    nc.gpsimd.collective_compute(
                        'ReduceScatter', OP.add,
                        ins=[rs_in[0:1024, :].opt()],
                        outs=[rs_out[0:512, :].opt()],
                        replica_groups=grp)
                    for tk in tasks:
                        run_task(tk)
                    nc.gpsimd.collective_compute(
                        'ReduceScatter', OP.add,
                        ins=[rs_in[1024:2048, :].opt()],
                        outs=[rs_out[512:1024, :].opt()],
                        replica_groups=grp)

            # ====== phase D: residual + MLP over my T/2 tokens ======
            with contextlib.ExitStack() as pd:
                d_t = pd.enter_context(tc.tile_pool(name='d_t', bufs=2))
                d_big = pd.enter_context(tc.tile_pool(name='d_big', bufs=1))
                x2_sb = d_big.tile([128, NH, C], f32)
                xn2T = d_big.tile([128, 10, T // 2], bf16)
                h2T = d_big.tile([128, F // 128, T // 2], bf16)
                rinv_sb = d_big.tile([128, NH], f32)

                with tc.tile_pool(name='d_ps', bufs=4, space='PSUM') as d_ps, \
                     tc.tile_pool(name='dt_ps', bufs=2,
                                  space='PSUM') as dt_ps:
                    for tt in range(NH):
                        rsx = d_t.tile([128, C], bf16, tag='rsx')
                        nc.sync.dma_start(
                            out=rsx[:],
                            in_=rs_out[tt * 128:(tt + 1) * 128, :])
                        xht = d_t.tile([128, C], f32, tag='xht')
                        nc.sync.dma_start(
                            out=xht[:],
                            in_=xh_d[tt * 128:(tt + 1) * 128, :])
                        nc.vector.tensor_add(out=x2_sb[:, tt, :],
                                             in0=rsx[:], in1=xht[:])
                        ssq2 = d_t.tile([128, 1], f32, tag='ssq2')
                        nc.scalar.activation(out=scr_sq[:],
                                             in_=x2_sb[:, tt, :],
                                             func=AF.Square, bias=zero_sb[:],
                                             accum_out=ssq2[:])
                        m2 = d_t.tile([128, 1], f32, tag='m2')
                        nc.vector.tensor_scalar(out=m2[:], in0=ssq2[:],
                                                scalar1=float(1.0 / C),
                                                scalar2=EPS,
                                                op0=OP.mult, op1=OP.add)
                        nc.vector.reciprocal_approx_fast(
                            out=rinv_sb[:, tt:tt + 1], in_=m2[:])
                        for jg, (lo, hi) in enumerate(((0, 4), (4, 8),
                                                      (8, 10))):
                            tp2 = dt_ps.tile([128, 512], f32, tag='tp2')
                            for j in range(lo, hi):
                                nc.tensor.transpose(
                                    tp2[:, (j - lo) * 128:(j - lo + 1) * 128],
                                    x2_sb[:, tt, j * 128:(j + 1) * 128],
                                    ident_f[:])
                            nc.vector.tensor_copy(
                                out=xn2T[:, lo:hi, tt * 128:(tt + 1) * 128],
                                in_=tp2[:, 0:(hi - lo) * 128].rearrange(
                                    'p (j d) -> p j d', d=128))

                    # fc1: j-loop with LDW amortized over two 512 chunks
                    with tc.tile_pool(name='wf1', bufs=5) as wf1_pool:
                        for fi in range(F // 128):
                            wf1 = wf1_pool.tile([128, 10, 128], bf16,
                                                tag='wf1')
                            nc.sync.dma_start(
                                out=wf1[:],
                                in_=wfc1_d[:, fi * 128:(fi + 1) * 128]
                                .rearrange('(j p) c -> p j c', p=128))
                            hp0 = d_ps.tile([128, 512], f32, tag='hps')
                            hp1 = d_ps.tile([128, 512], f32, tag='hps')
                            for j in range(10):
                                nc.tensor.matmul(hp0[:], wf1[:, j, :],
                                                 xn2T[:, j, 0:512],
                                                 start=(j == 0),
                                                 stop=(j == 9))
                                nc.tensor.matmul(hp1[:], wf1[:, j, :],
                                                 xn2T[:, j, 512:1024],
                                                 start=(j == 0),
                                                 stop=(j == 9))
                            for ci, hp in ((0, hp0), (1, hp1)):
                                hrelu = d_t.tile([128, 512], bf16,
                                                 tag='hrelu')
                                nc.scalar.activation(out=hrelu[:], in_=hp[:],
                                                     func=AF.Relu,
                                                     bias=zero_sb[:])
                                nc.vector.tensor_mul(
                                    out=h2T[:, fi,
                                            ci * 512:(ci + 1) * 512],
                                    in0=hrelu[:], in1=hrelu[:])

                # fc2: c-halves x tt-quads; 2 matmuls (640 cols) per lhsT
                with tc.tile_pool(name='y_ps', bufs=4, space='PSUM') as y_ps, \
                     tc.tile_pool(name='wf2', bufs=3) as wf2_pool:
                    for clo, chi in ((0, 640), (640, C)):
                        for ttg in range(2):
                            yps = [y_ps.tile([128, 640], f32, tag='yps',
                                             name='yps')
                                   for _ in range(4)]
                            for f2 in range(F // 256):
                                wf2 = wf2_pool.tile([128, 2, 640], bf16,
                                                    tag='wf2')
                                nc.sync.dma_start(
                                    out=wf2[:],
                                    in_=wfc2_d[f2 * 256:(f2 + 1) * 256,
                                               clo:chi]
                                    .rearrange('(u p) c -> p u c', p=128))
                                for u in range(2):
                                    fi = 2 * f2 + u
                                    st = (fi == 0)
                                    sp_ = (fi == F // 128 - 1)
                                    for i in range(4):
                                        tt = 4 * ttg + i
                                        lhsT = h2T[:, fi,
                                                   tt * 128:(tt + 1) * 128]
                                        nc.tensor.matmul(
                                            yps[i][:, 0:512], lhsT,
                                            wf2[:, u, 0:512],
                                            start=st, stop=sp_)
                                        nc.tensor.matmul(
                                            yps[i][:, 512:640], lhsT,
                                            wf2[:, u, 512:640],
                                            start=st, stop=sp_)
                            for i in range(4):
                                tt = 4 * ttg + i
                                yo = d_t.tile([128, 640], f32, tag='yo')
                                nc.vector.scalar_tensor_tensor(
                                    out=yo[:], in0=yps[i][:],
                                    scalar=rinv_sb[:, tt:tt + 1],
                                    in1=x2_sb[:, tt, clo:chi],
                                    op0=OP.mult, op1=OP.add)
                                nc.sync.dma_start(
                                    out=y_d[tt * 128:(tt + 1) * 128,
                                            clo:chi],
                                    in_=yo[:])

    nc.compile()
    return nc


_CACHE = {}


def _get_nc(t_len=T):
    if t_len not in _CACHE:
        _CACHE[t_len] = build_nc(t_len)
    return _CACHE[t_len]


def make_in_maps(x, rotary_pos_emb, ln1_w, w_qkv, qn_w, kn_w, w_out, ln2_w,
                 w_fc1, w_fc2, t_len=T):
    """Host-side sharding prep. Returns list of per-core input dicts."""
    x = np.asarray(x, np.float32)
    rot = np.asarray(rotary_pos_emb, np.float32)
    cos = np.cos(rot).astype(np.float32)
    sin = np.sin(rot).astype(np.float32)
    sinneg = np.concatenate([-sin[:, :64], sin[:, :64]], axis=-1)
    qn = np.asarray(qn_w, np.float32)
    kn = np.asarray(kn_w, np.float32)
    cosq = (cos * qn).astype(ml_dtypes.bfloat16)
    sinq = (sinneg * qn).astype(ml_dtypes.bfloat16)
    cosk = (cos * kn).astype(ml_dtypes.bfloat16)
    sink = (sinneg * kn).astype(ml_dtypes.bfloat16)
    w_qkv_f = (np.asarray(w_qkv, np.float32)
               * np.asarray(ln1_w, np.float32)[:, None]).reshape(C, 3, H, D)
    w_fc1_f = (np.asarray(w_fc1, np.float32)
               * np.asarray(ln2_w, np.float32)[:, None]
               ).astype(ml_dtypes.bfloat16)
    w_fc2_b = np.asarray(w_fc2, np.float32).astype(ml_dtypes.bfloat16)
    wo = np.asarray(w_out, np.float32).reshape(H, D, C)

    in_maps = []
    for c in range(N_CORES):
        b, hg = c // 2, c % 2
        heads = slice(hg * HPC, (hg + 1) * HPC)
        wq = np.ascontiguousarray(
            w_qkv_f[:, :, heads, :].reshape(C, 3 * CPC)
        ).astype(ml_dtypes.bfloat16)
        w_outp = np.ascontiguousarray(
            wo[heads].reshape(CPC, C)).astype(ml_dtypes.bfloat16)
        in_maps.append({
            'x': np.ascontiguousarray(x[b]),
            'xh': np.ascontiguousarray(x[b, hg * T // 2:(hg + 1) * T // 2]),
            'w_qkv': wq,
            'cosq': cosq, 'sinq': sinq, 'cosk': cosk, 'sink': sink,
            'w_out': w_outp,
            'w_fc1': np.ascontiguousarray(w_fc1_f),
            'w_fc2': np.ascontiguousarray(w_fc2_b),
        })
    return in_maps


def assemble_output(results, t_len=T):
    out = np.zeros((B, t_len, C), np.float32)
    for c in range(N_CORES):
        b, hg = c // 2, c % 2
        out[b, hg * t_len // 2:(hg + 1) * t_len // 2] = results[c]['y']
    return out


def kernel(**inputs):
    nc = _get_nc(T)
    in_maps = make_in_maps(**inputs)
    res = bass_utils.run_bass_kernel_spmd(nc, in_maps,
                                          core_ids=list(range(N_CORES)))
    return assemble_output(res.results)


# revision 23
# speedup vs baseline: 1.1171x; 1.1171x over previous
"""Trainium2 Bass kernel for nn_Block_27848567948000 (dense transformer block).

Sharding (8 NeuronCores): 4 data-parallel groups over batch (B=4), 2-way
tensor-parallel within each pair: attention sharded over heads (5 each).
out_proj computed as per-head partial sums over ALL T, summed + token-scattered
via a pairwise ReduceScatter; MLP over the core's T/2 token half.

kernel(**inputs) takes FULL inputs and returns the FULL (4, 2048, 1280) output.
"""
import sys

sys.path.insert(0, '/opt/trn_rl_repo')

import numpy as np
import ml_dtypes

import concourse.bass as bass
import concourse.tile as tile
from concourse import mybir, bacc
from concourse import bass_utils
from concourse.masks import make_identity

B, T, C, H, D, F = 4, 2048, 1280, 10, 128, 5120
EPS = 1e-5
N_CORES = 8
HPC = H // 2            # heads per core (5)
CPC = HPC * D           # channels per core (640)
f32 = mybir.dt.float32
bf16 = mybir.dt.bfloat16
fp8 = mybir.dt.float8e4
i32 = mybir.dt.int32
AF = mybir.ActivationFunctionType
OP = mybir.AluOpType
AX = mybir.AxisListType

NT = T // 128            # 16 token tiles
NH = T // 2 // 128       # 8 token tiles in my half
QB = T // 512            # 4 query blocks
NBLK = HPC * 4           # 20 mxfp8 blocks per tensor per token
INV_SQRT_D = float(1.0 / np.sqrt(D))
NEG = -30000.0


def _ap(t_ap, offset_delta, pattern):
    return bass.AP(tensor=t_ap.tensor, offset=t_ap.offset + offset_delta,
                   ap=pattern)


def _rsqrt_vec(nc, pool, out_ap, in_ap, scale, eps, tag, eng=None):
    """out = 1/sqrt(in*scale + eps) on a DVE-like engine (no act tables).
    Bit-trick seed + 2 Newton iterations (~1e-6 rel err). Shapes (128, n)."""
    if eng is None:
        eng = nc.vector
    i32_ = mybir.dt.int32
    shp = [128, in_ap.free_size()]
    m = pool.tile(shp, f32, tag=tag + 'm', name='rs_m')
    eng.tensor_scalar(out=m[:], in0=in_ap, scalar1=scale, scalar2=eps,
                      op0=OP.mult, op1=OP.add)
    y = pool.tile(shp, f32, tag=tag + 'y', name='rs_y')
    eng.tensor_single_scalar(out=y[:].bitcast(i32_),
                             in_=m[:].bitcast(i32_), scalar=1,
                             op=OP.logical_shift_right)
    eng.tensor_scalar(out=y[:].bitcast(i32_), in0=y[:].bitcast(i32_),
                      scalar1=-1, scalar2=0x5f3759df,
                      op0=OP.mult, op1=OP.add)
    t = pool.tile(shp, f32, tag=tag + 't', name='rs_t')
    for it in range(2):
        eng.tensor_tensor(out=t[:], in0=y[:], in1=y[:], op=OP.mult)
        eng.tensor_tensor(out=t[:], in0=t[:], in1=m[:], op=OP.mult)
        eng.tensor_scalar(out=t[:], in0=t[:], scalar1=-0.5,
                          scalar2=1.5, op0=OP.mult, op1=OP.add)
        eng.tensor_tensor(out=y[:] if it == 0 else out_ap, in0=y[:],
                          in1=t[:], op=OP.mult)


def build_nc(t_len=T, n_cores=N_CORES):
    import contextlib
    nc = bacc.Bacc('TRN2', target_bir_lowering=False, debug=False,
                   num_devices=n_cores)

    # ---- DRAM I/O ----
    x_d = nc.dram_tensor('x', [T, C], f32, kind='ExternalInput')
    xh_d = nc.dram_tensor('xh', [T // 2, C], f32, kind='ExternalInput')
    wqkv_d = nc.dram_tensor('w_qkv', [C, 3 * CPC], bf16, kind='ExternalInput')
    cosq_d = nc.dram_tensor('cosq', [T, D], bf16, kind='ExternalInput')
    sinq_d = nc.dram_tensor('sinq', [T, D], bf16, kind='ExternalInput')
    cosk_d = nc.dram_tensor('cosk', [T, D], bf16, kind='ExternalInput')
    sink_d = nc.dram_tensor('sink', [T, D], bf16, kind='ExternalInput')
    wout_d = nc.dram_tensor('w_out', [CPC, C], bf16, kind='ExternalInput')
    wfc1_d = nc.dram_tensor('w_fc1', [C, F], bf16, kind='ExternalInput')
    wfc2_d = nc.dram_tensor('w_fc2', [F, C], bf16, kind='ExternalInput')
    y_d = nc.dram_tensor('y', [T // 2, C], f32, kind='ExternalOutput')

    with tile.TileContext(nc) as tc:
        with contextlib.ExitStack() as ctx:
            persist = ctx.enter_context(tc.tile_pool(name='persist', bufs=1))
            dram = ctx.enter_context(tc.tile_pool(name='dram', bufs=1,
                                                  space='DRAM'))

            # ---- constants ----
            ident_b = persist.tile([128, 128], bf16)
            make_identity(nc, ident_b)
            ident_f = persist.tile([128, 128], f32)
            make_identity(nc, ident_f)
            ones128 = persist.tile([128, 128], bf16)
            nc.vector.memset(ones128[:], 1.0)
            zero_sb = persist.tile([128, 1], f32)
            nc.vector.memset(zero_sb[:], 0.0)
            eps_sb = persist.tile([128, 1], f32)
            nc.vector.memset(eps_sb[:], EPS)
            scr_sq = persist.tile([128, C], bf16)   # Square-output scratch

            # DRAM scratch for the collective
            rs_in = dram.tile([T, C], bf16)
            rs_out = dram.tile([T // 2, C], bf16)

            with contextlib.ExitStack() as pab:
                ab = pab.enter_context(tc.tile_pool(name='ab', bufs=1))
                qT = ab.tile([128, HPC, T], bf16)
                kT = ab.tile([128, HPC, T], bf16)
                vd_sb = ab.tile([128, NT, HPC, 130], bf16)
                attnT = ab.tile([128, HPC, T], bf16)
                nc.vector.memset(vd_sb[:, :, :, 128:129], 1.0)

                # ====== phases A+B ======
                with contextlib.ExitStack() as pin:
                    a_w = pin.enter_context(tc.tile_pool(name='a_w', bufs=1))
                    wq_sb = a_w.tile([128, 10, 3 * CPC], bf16)
                    nc.sync.dma_start(
                        out=wq_sb[:],
                        in_=wqkv_d.ap().rearrange('(j p) c -> p j c', p=128))
                    cq_sb = a_w.tile([128, NT, D], bf16)
                    nc.sync.dma_start(
                        out=cq_sb[:],
                        in_=cosq_d.ap().rearrange('(t p) d -> p t d', p=128))
                    sq_sb = a_w.tile([128, NT, D], bf16)
                    nc.sync.dma_start(
                        out=sq_sb[:],
                        in_=sinq_d.ap().rearrange('(t p) d -> p t d', p=128))
                    ck_sb = a_w.tile([128, NT, D], bf16)
                    nc.sync.dma_start(
                        out=ck_sb[:],
                        in_=cosk_d.ap().rearrange('(t p) d -> p t d', p=128))
                    sk_sb = a_w.tile([128, NT, D], bf16)
                    nc.sync.dma_start(
                        out=sk_sb[:],
                        in_=sink_d.ap().rearrange('(t p) d -> p t d', p=128))

                    a_t = pin.enter_context(tc.tile_pool(name='a_t', bufs=2))
                    a_s = pin.enter_context(tc.tile_pool(name='a_s', bufs=2))
                    a_q = pin.enter_context(tc.tile_pool(name='a_q', bufs=2))
                    pT_pool = pin.enter_context(
                        tc.tile_pool(name='pT', bufs=3))
                    b_t = pin.enter_context(tc.tile_pool(name='b_t', bufs=2))
                    ps512 = pin.enter_context(
                        tc.tile_pool(name='ps512', bufs=3, space='PSUM'))
                    ops_ps = pin.enter_context(
                        tc.tile_pool(name='ops_ps', bufs=1, space='PSUM'))
                    psT = pin.enter_context(
                        tc.tile_pool(name='psT', bufs=3, space='PSUM'))
                    psD = pin.enter_context(
                        tc.tile_pool(name='psD', bufs=1, space='PSUM'))

                    stash = {}

                    def emit_head(t):
                        xt = a_s.tile([128, C], f32, tag='xt')
                        nc.sync.dma_start(out=xt[:],
                                          in_=x_d[t * 128:(t + 1) * 128, :])
                        ssq = a_s.tile([128, 1], f32, tag='ssq')
                        nc.scalar.activation(out=scr_sq[:], in_=xt[:],
                                             func=AF.Square, bias=zero_sb[:],
                                             accum_out=ssq[:])
                        rstd = a_s.tile([128, 1], f32, tag='rstd')
                        _rsqrt_vec(nc, a_s, rstd[:], ssq[:],
                                   float(1.0 / C), EPS, 'rx')
                        xnT = a_s.tile([128, 10, 128], bf16, tag='xnT')
                        for jg, (lo, hi) in enumerate(((0, 4), (4, 8),
                                                      (8, 10))):
                            tp = psT.tile([128, 512], f32, tag='tp',
                                          name='tpf')
                            for j in range(lo, hi):
                                nc.tensor.transpose(
                                    tp[:, (j - lo) * 128:(j - lo + 1) * 128],
                                    xt[:, j * 128:(j + 1) * 128], ident_f[:])
                            nc.scalar.copy(
                                out=xnT[:, lo:hi, :],
                                in_=tp[:, 0:(hi - lo) * 128].rearrange(
                                    'p (j d) -> p j d', d=128))
                        # QKV (chunk-outer, j-mid, g-inner: LDW amortized)
                        qf = a_q.tile([128, CPC], bf16, tag='qf')
                        kf = a_q.tile([128, CPC], bf16, tag='kf')
                        vf = a_q.tile([128, CPC], bf16, tag='vf')
                        dsts = (qf, kf, vf)
                        for lo, hi in ((0, 512), (512, 640)):
                            pss = [ps512.tile([128, 512], f32, tag='mm',
                                              name='qkvps')
                                   for _ in range(3)]
                            for j in range(10):
                                for g in range(3):
                                    nc.tensor.matmul(
                                        pss[g][:, 0:hi - lo], xnT[:, j, :],
                                        wq_sb[:, j,
                                              g * CPC + lo:g * CPC + hi],
                                        start=(j == 0), stop=(j == 9))
                            for g in range(3):
                                if g == 2:
                                    nc.vector.tensor_scalar_mul(
                                        out=dsts[g][:, lo:hi],
                                        in0=pss[g][:, 0:hi - lo],
                                        scalar1=rstd[:])
                                else:
                                    nc.scalar.activation(
                                        out=dsts[g][:, lo:hi],
                                        in_=pss[g][:, 0:hi - lo],
                                        func=AF.Copy, scale=rstd[:])
                        stash[t] = (qf, kf, vf)

                    def rope(eng, src, cos_t, sin_t, out):
                        # out[p,h,d] = src*cos + swap(src)*sinneg   (bf16)
                        src3 = src[:].rearrange('p (h d) -> p h d', h=HPC)
                        pa = list(src3.ap)
                        swap = _ap(src3, 64, pa[:2] + [[-64, 2], [1, 64]])
                        ca = list(cos_t.ap)
                        cos4 = _ap(cos_t, 0, [ca[0], [0, HPC], [1, 128]])
                        sin4 = _ap(sin_t, 0,
                                   [ca[0], [0, HPC], [64, 2], [1, 64]])
                        tmp = a_t.tile([128, HPC, D], bf16, tag='rtmp')
                        eng.tensor_tensor(
                            out=tmp[:].rearrange('p h (u d) -> p h u d', u=2),
                            in0=swap, in1=sin4, op=OP.mult)
                        eng.tensor_tensor(out=out[:], in0=src3, in1=cos4,
                                          op=OP.mult)
                        eng.tensor_add(out=out[:], in0=out[:], in1=tmp[:])

                    def blk4(ap20, reps=32):
                        # (128,20) -> (128,5,4,reps) block broadcast
                        a = list(ap20.ap)
                        st = a[-1][0]
                        return bass.AP(tensor=ap20.tensor, offset=ap20.offset,
                                       ap=[a[0], [4 * st, HPC], [st, 4],
                                           [0, reps]])

                    def hb(ap5, reps=4):
                        # (128,5) -> (128,5,reps) broadcast
                        a = list(ap5.ap)
                        return bass.AP(tensor=ap5.tensor, offset=ap5.offset,
                                       ap=[a[0], [a[-1][0], HPC], [0, reps]])

                    def v4(x):
                        return x.rearrange('p h (b e) -> p h b e', e=32)

                    def emit_tail(t):
                        qf, kf, vf = stash.pop(t)
                        # rms of pre-rope q/k (rope is norm-preserving)
                        msq = a_t.tile([128, 2, HPC], f32, tag='msq')
                        for h in range(HPC):
                            nc.scalar.activation(
                                out=scr_sq[:, 0:D],
                                in_=qf[:, h * D:(h + 1) * D],
                                func=AF.Square, bias=zero_sb[:],
                                accum_out=msq[:, 0, h:h + 1])
                        ksq = a_t.tile([128, HPC, D], bf16, tag='ksq')
                        kf3 = kf[:].rearrange('p (h d) -> p h d', h=HPC)
                        nc.vector.tensor_tensor(out=ksq[:], in0=kf3,
                                                in1=kf3, op=OP.mult)
                        nc.vector.tensor_reduce(out=msq[:, 1, :],
                                                in_=ksq[:], axis=AX.X,
                                                op=OP.add)
                        _rsqrt_vec(nc, a_t, msq[:], msq[:],
                                   float(1.0 / D), EPS, 'rqk')
                        # rope (q on vector, k on gpsimd)
                        zq = a_t.tile([128, HPC, D], bf16, tag='zq')
                        rope(nc.vector, qf, cq_sb[:, t, :], sq_sb[:, t, :],
                             zq)
                        zk = a_t.tile([128, HPC, D], bf16, tag='zk')
                        rope(nc.gpsimd, kf, ck_sb[:, t, :], sk_sb[:, t, :],
                             zk)
                        # block amax; amn = amax*rstd (q,k) or amax (v)
                        amn = a_t.tile([128, 3, NBLK], f32, tag='amn')
                        nc.vector.tensor_reduce(
                            out=amn[:, 0, :], in_=v4(zq[:]), axis=AX.X,
                            op=OP.max, apply_absolute_value=True)
                        nc.vector.tensor_reduce(
                            out=amn[:, 1, :], in_=v4(zk[:]), axis=AX.X,
                            op=OP.max, apply_absolute_value=True)
                        nc.vector.tensor_reduce(
                            out=amn[:, 2, :],
                            in_=vf[:].rearrange('p (h b e) -> p h b e',
                                                h=HPC, e=32),
                            axis=AX.X, op=OP.max, apply_absolute_value=True)
                        for i in range(2):
                            nc.vector.tensor_tensor(
                                out=amn[:, i, :].rearrange(
                                    'p (h b) -> p h b', h=HPC),
                                in0=amn[:, i, :].rearrange(
                                    'p (h b) -> p h b', h=HPC),
                                in1=hb(msq[:, i, :]), op=OP.mult)
                        nc.vector.tensor_scalar_max(out=amn[:], in0=amn[:],
                                                    scalar1=1e-12)
                        eb = a_t.tile([128, 3, NBLK], i32, tag='eb')
                        nc.vector.tensor_single_scalar(
                            out=eb[:], in_=amn[:].bitcast(i32), scalar=23,
                            op=OP.logical_shift_right)
                        sc = a_t.tile([128, 3, NBLK], f32, tag='sc')
                        nc.vector.tensor_scalar(
                            out=sc[:].bitcast(i32), in0=eb[:], scalar1=-1,
                            scalar2=260, op0=OP.mult, op1=OP.add)
                        nc.vector.tensor_single_scalar(
                            out=sc[:].bitcast(i32), in_=sc[:].bitcast(i32),
                            scalar=23, op=OP.logical_shift_left)
                        isc = a_t.tile([128, 3, NBLK], f32, tag='isc')
                        nc.vector.tensor_single_scalar(
                            out=isc[:].bitcast(i32), in_=eb[:], scalar=6,
                            op=OP.subtract)
                        nc.vector.tensor_single_scalar(
                            out=isc[:].bitcast(i32), in_=isc[:].bitcast(i32),
                            scalar=23, op=OP.logical_shift_left)
                        msc = a_t.tile([128, 2, NBLK], f32, tag='msc')
                        for i in range(2):
                            nc.vector.tensor_tensor(
                                out=msc[:, i, :].rearrange(
                                    'p (h b) -> p h b', h=HPC),
                                in0=sc[:, i, :].rearrange(
                                    'p (h b) -> p h b', h=HPC),
                                in1=hb(msq[:, i, :]), op=OP.mult)
                        # quantize q (vector)
                        ys = a_t.tile([128, HPC, D], bf16, tag='ys')
                        q8 = a_t.tile([128, HPC, D], fp8, tag='q8')
                        qd = a_t.tile([128, HPC, D], bf16, tag='qd')
                        nc.vector.tensor_tensor(out=v4(ys[:]), in0=v4(zq[:]),
                                                in1=blk4(msc[:, 0, :]),
                                                op=OP.mult)
                        nc.vector.tensor_scalar(out=q8[:], in0=ys[:],
                                                scalar1=-112.0,
                                                scalar2=112.0,
                                                op0=OP.max, op1=OP.min)
                        nc.vector.tensor_tensor(out=v4(qd[:]), in0=v4(q8[:]),
                                                in1=blk4(isc[:, 0, :]),
                                                op=OP.mult)
                        # quantize k (gpsimd mults, vector fp8 cast)
                        ysk = a_t.tile([128, HPC, D], bf16, tag='ys')
                        k8 = a_t.tile([128, HPC, D], fp8, tag='q8')
                        kd = a_t.tile([128, HPC, D], bf16, tag='kd')
                        nc.gpsimd.tensor_tensor(out=v4(ysk[:]),
                                                in0=v4(zk[:]),
                                                in1=blk4(msc[:, 1, :]),
                                                op=OP.mult)
                        nc.vector.tensor_scalar(out=k8[:], in0=ysk[:],
                                                scalar1=-112.0,
                                                scalar2=112.0,
                                                op0=OP.max, op1=OP.min)
                        nc.gpsimd.tensor_tensor(out=v4(kd[:]), in0=v4(k8[:]),
                                                in1=blk4(isc[:, 1, :]),
                                                op=OP.mult)
                        # quantize v (vector; deq straight into vd_sb)
                        ysv = a_t.tile([128, HPC, D], bf16, tag='ys')
                        v8 = a_t.tile([128, HPC, D], fp8, tag='q8')
                        nc.vector.tensor_tensor(
                            out=v4(ysv[:]),
                            in0=v4(vf[:].rearrange('p (h d) -> p h d',
                                                   h=HPC)),
                            in1=blk4(sc[:, 2, :]), op=OP.mult)
                        nc.vector.tensor_scalar(out=v8[:], in0=ysv[:],
                                                scalar1=-112.0,
                                                scalar2=112.0,
                                                op0=OP.max, op1=OP.min)
                        nc.vector.tensor_tensor(
                            out=v4(vd_sb[:, t, :, 0:D]), in0=v4(v8[:]),
                            in1=blk4(isc[:, 2, :]), op=OP.mult)
                        # transpose qd/kd into qT/kT
                        for src, dstT in ((qd, qT), (kd, kT)):
                            tp = psT.tile([128, 640], bf16, tag='tp')
                            for h in range(HPC):
                                nc.tensor.transpose(
                                    tp[:, h * 128:(h + 1) * 128],
                                    src[:, h, :], ident_b[:])
                            nc.vector.tensor_copy(
                                out=dstT[:, :, t * 128:(t + 1) * 128],
                                in_=tp[:].rearrange('p (h d) -> p h d',
                                                    h=HPC))

                    def emit_attn_h(qb, h):
                        nkt = 4 * qb + 4
                        if True:
                            dps = psD.tile([128, 512], f32, tag='dps')
                            ops = ops_ps.tile([128, 512], f32, tag='ops')
                            for kt in range(nkt):
                                sp = ps512.tile([128, 512], f32, tag='mm')
                                o = kt - 4 * qb
                                nc.tensor.matmul(
                                    sp[:],
                                    kT[:, h, kt * 128:(kt + 1) * 128],
                                    qT[:, h, qb * 512:(qb + 1) * 512],
                                    start=True, stop=True)
                                pT = pT_pool.tile([128, 512], bf16, tag='pT')
                                nc.scalar.activation(out=pT[:], in_=sp[:],
                                                     func=AF.Exp,
                                                     bias=zero_sb[:],
                                                     scale=INV_SQRT_D)
                                if o >= 0:
                                    nc.gpsimd.affine_select(
                                        out=pT[:], in_=pT[:],
                                        compare_op=OP.is_ge, fill=0.0,
                                        base=-128 * o, pattern=[[1, 512]],
                                        channel_multiplier=-1)
                                nc.tensor.matmul(dps[:], ones128[:], pT[:],
                                                 start=(kt == 0),
                                                 stop=(kt == nkt - 1))
                                nc.tensor.matmul(ops[:],
                                                 vd_sb[:, kt, h, 0:128],
                                                 pT[:],
                                                 start=(kt == 0),
                                                 stop=(kt == nkt - 1))
                            rd = b_t.tile([128, 512], f32, tag='rd')
                            nc.vector.reciprocal_approx_fast(out=rd[:],
                                                             in_=dps[:])
                            nc.vector.tensor_tensor(
                                out=attnT[:, h, qb * 512:(qb + 1) * 512],
                                in0=ops[:], in1=rd[:], op=OP.mult)

                    wo_sb = a_w.tile([128, HPC, C], bf16)
                    nc.sync.dma_start(
                        out=wo_sb[:],
                        in_=wout_d.ap().rearrange('(h p) c -> p h c', p=128))
                    # rs_in row layout: [t0:512 | t1024:1536 | t512:1024
                    # | t1536:2048] so each RS half is contiguous.
                    rowblk = {tt: i for i, tt in enumerate(
                        (0, 1, 2, 3, 8, 9, 10, 11, 4, 5, 6, 7,
                         12, 13, 14, 15))}
                    grp = [[2 * i, 2 * i + 1] for i in range(n_cores // 2)]

                    def oproj(tt):
                        ob = b_t.tile([128, C], bf16, tag='ob', name='ob')
                        for ci, (lo, hi) in enumerate(((0, 512),
                                                       (512, 1024),
                                                       (1024, C))):
                            ps = ps512.tile([128, 512], f32, tag='mm',
                                            name='oprojps')
                            for h in range(HPC):
                                nc.tensor.matmul(
                                    ps[:, 0:hi - lo],
                                    attnT[:, h, tt * 128:(tt + 1) * 128],
                                    wo_sb[:, h, lo:hi],
                                    start=(h == 0), stop=(h == HPC - 1))
                            if ci == 2:
                                nc.scalar.copy(out=ob[:, lo:hi],
                                               in_=ps[:, 0:hi - lo])
                            else:
                                nc.vector.tensor_copy(
                                    out=ob[:, lo:hi], in_=ps[:, 0:hi - lo])
                        r = rowblk[tt]
                        nc.sync.dma_start(
                            out=rs_in[r * 128:(r + 1) * 128, :], in_=ob[:])

                    # ---- interleaved A+B+C emission ----
                    for t in range(NT):
                        emit_head(t)
                        if t >= 1:
                            emit_tail(t - 1)
                        if t >= 4 and t % 4 == 0:
                            qb = t // 4 - 1
                            for h in range(HPC):
                                emit_attn_h(qb, h)
                            for tt in range(4 * qb, 4 * qb + 4):
                                oproj(tt)
                            if qb == 2:
                                nc.gpsimd.collective_compute(
                                    'ReduceScatter', OP.add,
                                    ins=[rs_in[0:1024, :].opt()],
                                    outs=[rs_out[0:512, :].opt()],
                                    replica_groups=grp)
                    emit_tail(NT - 1)
                    for h in range(HPC):
                        emit_attn_h(QB - 1, h)
                    for tt in range(12, 16):
                        oproj(tt)
                    nc.gpsimd.collective_compute(
                        'ReduceScatter', OP.add,
                        ins=[rs_in[1024:2048, :].opt()],
                        outs=[rs_out[512:1024, :].opt()],
                        replica_groups=grp)

            # ====== phase D: residual + MLP over my T/2 tokens ======
            with contextlib.ExitStack() as pd:
                d_t = pd.enter_context(tc.tile_pool(name='d_t', bufs=2))
                d_big = pd.enter_context(tc.tile_pool(name='d_big', bufs=1))
                x2_sb = d_big.tile([128, NH, C], f32)
                xn2T = d_big.tile([128, 10, T // 2], bf16)
                h2T = d_big.tile([128, F // 128, T // 2], bf16)
                rinv_sb = d_big.tile([128, NH], f32)

                with tc.tile_pool(name='d_ps', bufs=4, space='PSUM') as d_ps, \
                     tc.tile_pool(name='dt_ps', bufs=2,
                                  space='PSUM') as dt_ps:
                    for tt in range(NH):
                        rsx = d_t.tile([128, C], bf16, tag='rsx')
                        nc.sync.dma_start(
                            out=rsx[:],
                            in_=rs_out[tt * 128:(tt + 1) * 128, :])
                        xht = d_t.tile([128, C], f32, tag='xht')
                        nc.sync.dma_start(
                            out=xht[:],
                            in_=xh_d[tt * 128:(tt + 1) * 128, :])
                        nc.vector.tensor_add(out=x2_sb[:, tt, :],
                                             in0=rsx[:], in1=xht[:])
                        ssq2 = d_t.tile([128, 1], f32, tag='ssq2')
                        nc.scalar.activation(out=scr_sq[:],
                                             in_=x2_sb[:, tt, :],
                                             func=AF.Square, bias=zero_sb[:],
                                             accum_out=ssq2[:])
                        m2 = d_t.tile([128, 1], f32, tag='m2')
                        nc.vector.tensor_scalar(out=m2[:], in0=ssq2[:],
                                                scalar1=float(1.0 / C),
                                                scalar2=EPS,
                                                op0=OP.mult, op1=OP.add)
                        nc.vector.reciprocal_approx_fast(
                            out=rinv_sb[:, tt:tt + 1], in_=m2[:])
                        for jg, (lo, hi) in enumerate(((0, 4), (4, 8),
                                                      (8, 10))):
                            tp2 = dt_ps.tile([128, 512], f32, tag='tp2')
                            for j in range(lo, hi):
                                nc.tensor.transpose(
                                    tp2[:, (j - lo) * 128:(j - lo + 1) * 128],
                                    x2_sb[:, tt, j * 128:(j + 1) * 128],
                                    ident_f[:])
                            nc.vector.tensor_copy(
                                out=xn2T[:, lo:hi, tt * 128:(tt + 1) * 128],
                                in_=tp2[:, 0:(hi - lo) * 128].rearrange(
                                    'p (j d) -> p j d', d=128))

                    # fc1: j-loop with LDW amortized over two 512 chunks
                    with tc.tile_pool(name='wf1', bufs=5) as wf1_pool:
                        for fi in range(F // 128):
                            wf1 = wf1_pool.tile([128, 10, 128], bf16,
                                                tag='wf1')
                            nc.sync.dma_start(
                                out=wf1[:],
                                in_=wfc1_d[:, fi * 128:(fi + 1) * 128]
                                .rearrange('(j p) c -> p j c', p=128))
                            hp0 = d_ps.tile([128, 512], f32, tag='hps')
                            hp1 = d_ps.tile([128, 512], f32, tag='hps')
                            for j in range(10):
                                nc.tensor.matmul(hp0[:], wf1[:, j, :],
                                                 xn2T[:, j, 0:512],
                                                 start=(j == 0),
                                                 stop=(j == 9))
                                nc.tensor.matmul(hp1[:], wf1[:, j, :],
                                                 xn2T[:, j, 512:1024],
                                                 start=(j == 0),
                                                 stop=(j == 9))
                            for ci, hp in ((0, hp0), (1, hp1)):
                                hrelu = d_t.tile([128, 512], bf16,
                                                 tag='hrelu')
                                nc.scalar.activation(out=hrelu[:], in_=hp[:],
                                                     func=AF.Relu,
                                                     bias=zero_sb[:])
                                nc.vector.tensor_mul(
                                    out=h2T[:, fi,
                                            ci * 512:(ci + 1) * 512],
                                    in0=hrelu[:], in1=hrelu[:])

                # fc2: c-halves x tt-quads; 2 matmuls (640 cols) per lhsT
                with tc.tile_pool(name='y_ps', bufs=4, space='PSUM') as y_ps, \
                     tc.tile_pool(name='wf2', bufs=3) as wf2_pool:
                    for clo, chi in ((0, 640), (640, C)):
                        for ttg in range(2):
                            yps = [y_ps.tile([128, 640], f32, tag='yps',
                                             name='yps')
                                   for _ in range(4)]
                            for f2 in range(F // 256):
                                wf2 = wf2_pool.tile([128, 2, 640], bf16,
                                                    tag='wf2')
                                nc.sync.dma_start(
                                    out=wf2[:],
                                    in_=wfc2_d[f2 * 256:(f2 + 1) * 256,
                                               clo:chi]
                                    .rearrange('(u p) c -> p u c', p=128))
                                for u in range(2):
                                    fi = 2 * f2 + u
                                    st = (fi == 0)
                                    sp_ = (fi == F // 128 - 1)
                                    for i in range(4):
                                        tt = 4 * ttg + i
                                        lhsT = h2T[:, fi,
                                                   tt * 128:(tt + 1) * 128]
                                        nc.tensor.matmul(
                                            yps[i][:, 0:512], lhsT,
                                            wf2[:, u, 0:512],
                                            start=st, stop=sp_)
                                        nc.tensor.matmul(
                                            yps[i][:, 512:640], lhsT,
                                            wf2[:, u, 512:640],
                                            start=st, stop=sp_)
                            for i in range(4):
                                tt = 4 * ttg + i
                                yo = d_t.tile([128, 640], f32, tag='yo')
                                nc.vector.scalar_tensor_tensor(
                                    out=yo[:], in0=yps[i][:],
                                    scalar=rinv_sb[:, tt:tt + 1],
                                    in1=x2_sb[:, tt, clo:chi],
                                    op0=OP.mult, op1=OP.add)
                                nc.sync.dma_start(
                                    out=y_d[tt * 128:(tt + 1) * 128,
                                            clo:chi],
                                    in_=yo[:])

    nc.compile()
    return nc


_CACHE = {}


def _get_nc(t_len=T):
    if t_len not in _CACHE:
        _CACHE[t_len] = build_nc(t_len)
    return _CACHE[t_len]


def make_in_maps(x, rotary_pos_emb, ln1_w, w_qkv, qn_w, kn_w, w_out, ln2_w,
                 w_fc1, w_fc2, t_len=T):
    """Host-side sharding prep. Returns list of per-core input dicts."""
    x = np.asarray(x, np.float32)
    rot = np.asarray(rotary_pos_emb, np.float32)
    cos = np.cos(rot).astype(np.float32)
    sin = np.sin(rot).astype(np.float32)
    sinneg = np.concatenate([-sin[:, :64], sin[:, :64]], axis=-1)
    qn = np.asarray(qn_w, np.float32)
    kn = np.asarray(kn_w, np.float32)
    cosq = (cos * qn).astype(ml_dtypes.bfloat16)
    sinq = (sinneg * qn).astype(ml_dtypes.bfloat16)
    cosk = (cos * kn).astype(ml_dtypes.bfloat16)
    sink = (sinneg * kn).astype(ml_dtypes.bfloat16)
    w_qkv_f = (np.asarray(w_qkv, np.float32)
               * np.asarray(ln1_w, np.float32)[:, None]).reshape(C, 3, H, D)
    w_fc1_f = (np.asarray(w_fc1, np.float32)
               * np.asarray(ln2_w, np.float32)[:, None]
               ).astype(ml_dtypes.bfloat16)
    w_fc2_b = np.asarray(w_fc2, np.float32).astype(ml_dtypes.bfloat16)
    wo = np.asarray(w_out, np.float32).reshape(H, D, C)

    in_maps = []
    for c in range(N_CORES):
        b, hg = c // 2, c % 2
        heads = slice(hg * HPC, (hg + 1) * HPC)
        wq = np.ascontiguousarray(
            w_qkv_f[:, :, heads, :].reshape(C, 3 * CPC)
        ).astype(ml_dtypes.bfloat16)
        w_outp = np.ascontiguousarray(
            wo[heads].reshape(CPC, C)).astype(ml_dtypes.bfloat16)
        in_maps.append({
            'x': np.ascontiguousarray(x[b]),
            'xh': np.ascontiguousarray(x[b, hg * T // 2:(hg + 1) * T // 2]),
            'w_qkv': wq,
            'cosq': cosq, 'sinq': sinq, 'cosk': cosk, 'sink': sink,
            'w_out': w_outp,
            'w_fc1': np.ascontiguousarray(w_fc1_f),
            'w_fc2': np.ascontiguousarray(w_fc2_b),
        })
    return in_maps


def assemble_output(results, t_len=T):
    out = np.zeros((B, t_len, C), np.float32)
    for c in range(N_CORES):
        b, hg = c // 2, c % 2
        out[b, hg * t_len // 2:(hg + 1) * t_len // 2] = results[c]['y']
    return out


def kernel(**inputs):
    nc = _get_nc(T)
    in_maps = make_in_maps(**inputs)
    res = bass_utils.run_bass_kernel_spmd(nc, in_maps,
                                          core_ids=list(range(N_CORES)))
    return assemble_output(res.results)


# revision 24
# speedup vs baseline: 1.1266x; 1.0086x over previous
"""Trainium2 Bass kernel for nn_Block_27848567948000 (dense transformer block).

Sharding (8 NeuronCores): 4 data-parallel groups over batch (B=4), 2-way
tensor-parallel within each pair: attention sharded over heads (5 each).
out_proj computed as per-head partial sums over ALL T, summed + token-scattered
via a pairwise ReduceScatter; MLP over the core's T/2 token half.

kernel(**inputs) takes FULL inputs and returns the FULL (4, 2048, 1280) output.
"""
import sys

sys.path.insert(0, '/opt/trn_rl_repo')

import numpy as np
import ml_dtypes

import concourse.bass as bass
import concourse.tile as tile
from concourse import mybir, bacc
from concourse import bass_utils
from concourse.masks import make_identity

B, T, C, H, D, F = 4, 2048, 1280, 10, 128, 5120
EPS = 1e-5
N_CORES = 8
HPC = H // 2            # heads per core (5)
CPC = HPC * D           # channels per core (640)
f32 = mybir.dt.float32
bf16 = mybir.dt.bfloat16
fp8 = mybir.dt.float8e4
i32 = mybir.dt.int32
AF = mybir.ActivationFunctionType
OP = mybir.AluOpType
AX = mybir.AxisListType

NT = T // 128            # 16 token tiles
NH = T // 2 // 128       # 8 token tiles in my half
QB = T // 512            # 4 query blocks
NBLK = HPC * 4           # 20 mxfp8 blocks per tensor per token
INV_SQRT_D = float(1.0 / np.sqrt(D))
NEG = -30000.0


def _ap(t_ap, offset_delta, pattern):
    return bass.AP(tensor=t_ap.tensor, offset=t_ap.offset + offset_delta,
                   ap=pattern)


def _rsqrt_vec(nc, pool, out_ap, in_ap, scale, eps, tag, eng=None):
    """out = 1/sqrt(in*scale + eps) on a DVE-like engine (no act tables).
    Bit-trick seed + 2 Newton iterations (~1e-6 rel err). Shapes (128, n)."""
    if eng is None:
        eng = nc.vector
    i32_ = mybir.dt.int32
    shp = [128, in_ap.free_size()]
    m = pool.tile(shp, f32, tag=tag + 'm', name='rs_m')
    eng.tensor_scalar(out=m[:], in0=in_ap, scalar1=scale, scalar2=eps,
                      op0=OP.mult, op1=OP.add)
    y = pool.tile(shp, f32, tag=tag + 'y', name='rs_y')
    eng.tensor_single_scalar(out=y[:].bitcast(i32_),
                             in_=m[:].bitcast(i32_), scalar=1,
                             op=OP.logical_shift_right)
    eng.tensor_scalar(out=y[:].bitcast(i32_), in0=y[:].bitcast(i32_),
                      scalar1=-1, scalar2=0x5f3759df,
                      op0=OP.mult, op1=OP.add)
    t = pool.tile(shp, f32, tag=tag + 't', name='rs_t')
    for it in range(2):
        eng.tensor_tensor(out=t[:], in0=y[:], in1=y[:], op=OP.mult)
        eng.tensor_tensor(out=t[:], in0=t[:], in1=m[:], op=OP.mult)
        eng.tensor_scalar(out=t[:], in0=t[:], scalar1=-0.5,
                          scalar2=1.5, op0=OP.mult, op1=OP.add)
        eng.tensor_tensor(out=y[:] if it == 0 else out_ap, in0=y[:],
                          in1=t[:], op=OP.mult)


def build_nc(t_len=T, n_cores=N_CORES):
    import contextlib
    nc = bacc.Bacc('TRN2', target_bir_lowering=False, debug=False,
                   num_devices=n_cores)

    # ---- DRAM I/O ----
    x_d = nc.dram_tensor('x', [T, C], f32, kind='ExternalInput')
    xh_d = nc.dram_tensor('xh', [T // 2, C], f32, kind='ExternalInput')
    wqkv_d = nc.dram_tensor('w_qkv', [C, 3 * CPC], bf16, kind='ExternalInput')
    cosq_d = nc.dram_tensor('cosq', [T, D], bf16, kind='ExternalInput')
    sinq_d = nc.dram_tensor('sinq', [T, D], bf16, kind='ExternalInput')
    cosk_d = nc.dram_tensor('cosk', [T, D], bf16, kind='ExternalInput')
    sink_d = nc.dram_tensor('sink', [T, D], bf16, kind='ExternalInput')
    wout_d = nc.dram_tensor('w_out', [CPC, C], bf16, kind='ExternalInput')
    wfc1_d = nc.dram_tensor('w_fc1', [C, F], bf16, kind='ExternalInput')
    wfc2_d = nc.dram_tensor('w_fc2', [F, C], bf16, kind='ExternalInput')
    y_d = nc.dram_tensor('y', [T // 2, C], f32, kind='ExternalOutput')

    with tile.TileContext(nc) as tc:
        with contextlib.ExitStack() as ctx:
            persist = ctx.enter_context(tc.tile_pool(name='persist', bufs=1))
            dram = ctx.enter_context(tc.tile_pool(name='dram', bufs=1,
                                                  space='DRAM'))

            # ---- constants ----
            ident_b = persist.tile([128, 128], bf16)
            make_identity(nc, ident_b)
            ident_f = persist.tile([128, 128], f32)
            make_identity(nc, ident_f)
            ones128 = persist.tile([128, 128], bf16)
            nc.vector.memset(ones128[:], 1.0)
            zero_sb = persist.tile([128, 1], f32)
            nc.vector.memset(zero_sb[:], 0.0)
            eps_sb = persist.tile([128, 1], f32)
            nc.vector.memset(eps_sb[:], EPS)
            scr_sq = persist.tile([128, C], bf16)   # Square-output scratch

            # DRAM scratch for the collective
            rs_in = dram.tile([T, C], bf16)
            rs_out = dram.tile([T // 2, C], bf16)

            with contextlib.ExitStack() as pab:
                ab = pab.enter_context(tc.tile_pool(name='ab', bufs=1))
                qT = ab.tile([128, HPC, T], bf16)
                kT = ab.tile([128, HPC, T], bf16)
                vd_sb = ab.tile([128, NT, HPC, 130], bf16)
                attnT = ab.tile([128, HPC, T], bf16)
                nc.vector.memset(vd_sb[:, :, :, 128:129], 1.0)

                # ====== phases A+B ======
                with contextlib.ExitStack() as pin:
                    a_w = pin.enter_context(tc.tile_pool(name='a_w', bufs=1))
                    wq_sb = a_w.tile([128, 10, 3 * CPC], bf16)
                    nc.sync.dma_start(
                        out=wq_sb[:],
                        in_=wqkv_d.ap().rearrange('(j p) c -> p j c', p=128))
                    cq_sb = a_w.tile([128, NT, D], bf16)
                    nc.sync.dma_start(
                        out=cq_sb[:],
                        in_=cosq_d.ap().rearrange('(t p) d -> p t d', p=128))
                    sq_sb = a_w.tile([128, NT, D], bf16)
                    nc.sync.dma_start(
                        out=sq_sb[:],
                        in_=sinq_d.ap().rearrange('(t p) d -> p t d', p=128))
                    ck_sb = a_w.tile([128, NT, D], bf16)
                    nc.sync.dma_start(
                        out=ck_sb[:],
                        in_=cosk_d.ap().rearrange('(t p) d -> p t d', p=128))
                    sk_sb = a_w.tile([128, NT, D], bf16)
                    nc.sync.dma_start(
                        out=sk_sb[:],
                        in_=sink_d.ap().rearrange('(t p) d -> p t d', p=128))

                    a_t = pin.enter_context(tc.tile_pool(name='a_t', bufs=2))
                    a_s = pin.enter_context(tc.tile_pool(name='a_s', bufs=2))
                    a_q = pin.enter_context(tc.tile_pool(name='a_q', bufs=2))
                    pT_pool = pin.enter_context(
                        tc.tile_pool(name='pT', bufs=4))
                    b_t = pin.enter_context(tc.tile_pool(name='b_t', bufs=2))
                    ps512 = pin.enter_context(
                        tc.tile_pool(name='ps512', bufs=3, space='PSUM'))
                    ops_ps = pin.enter_context(
                        tc.tile_pool(name='ops_ps', bufs=1, space='PSUM'))
                    psT = pin.enter_context(
                        tc.tile_pool(name='psT', bufs=3, space='PSUM'))
                    psD = pin.enter_context(
                        tc.tile_pool(name='psD', bufs=1, space='PSUM'))

                    stash = {}

                    def emit_head(t):
                        xt = a_s.tile([128, C], f32, tag='xt')
                        nc.sync.dma_start(out=xt[:],
                                          in_=x_d[t * 128:(t + 1) * 128, :])
                        ssq = a_s.tile([128, 1], f32, tag='ssq')
                        nc.scalar.activation(out=scr_sq[:], in_=xt[:],
                                             func=AF.Square, bias=zero_sb[:],
                                             accum_out=ssq[:])
                        rstd = a_s.tile([128, 1], f32, tag='rstd')
                        _rsqrt_vec(nc, a_s, rstd[:], ssq[:],
                                   float(1.0 / C), EPS, 'rx')
                        xnT = a_s.tile([128, 10, 128], bf16, tag='xnT')
                        for jg, (lo, hi) in enumerate(((0, 4), (4, 8),
                                                      (8, 10))):
                            tp = psT.tile([128, 512], f32, tag='tp',
                                          name='tpf')
                            for j in range(lo, hi):
                                nc.tensor.transpose(
                                    tp[:, (j - lo) * 128:(j - lo + 1) * 128],
                                    xt[:, j * 128:(j + 1) * 128], ident_f[:])
                            nc.scalar.copy(
                                out=xnT[:, lo:hi, :],
                                in_=tp[:, 0:(hi - lo) * 128].rearrange(
                                    'p (j d) -> p j d', d=128))
                        # QKV (chunk-outer, j-mid, g-inner: LDW amortized)
                        qf = a_q.tile([128, CPC], bf16, tag='qf')
                        kf = a_q.tile([128, CPC], bf16, tag='kf')
                        vf = a_q.tile([128, CPC], bf16, tag='vf')
                        dsts = (qf, kf, vf)
                        for lo, hi in ((0, 512), (512, 640)):
                            pss = [ps512.tile([128, 512], f32, tag='mm',
                                              name='qkvps')
                                   for _ in range(3)]
                            for j in range(10):
                                for g in range(3):
                                    nc.tensor.matmul(
                                        pss[g][:, 0:hi - lo], xnT[:, j, :],
                                        wq_sb[:, j,
                                              g * CPC + lo:g * CPC + hi],
                                        start=(j == 0), stop=(j == 9))
                            for g in range(3):
                                if g == 2:
                                    nc.vector.tensor_scalar_mul(
                                        out=dsts[g][:, lo:hi],
                                        in0=pss[g][:, 0:hi - lo],
                                        scalar1=rstd[:])
                                else:
                                    nc.scalar.activation(
                                        out=dsts[g][:, lo:hi],
                                        in_=pss[g][:, 0:hi - lo],
                                        func=AF.Copy, scale=rstd[:])
                        stash[t] = (qf, kf, vf)

                    def rope(eng, src, cos_t, sin_t, out):
                        # out[p,h,d] = src*cos + swap(src)*sinneg   (bf16)
                        src3 = src[:].rearrange('p (h d) -> p h d', h=HPC)
                        pa = list(src3.ap)
                        swap = _ap(src3, 64, pa[:2] + [[-64, 2], [1, 64]])
                        ca = list(cos_t.ap)
                        cos4 = _ap(cos_t, 0, [ca[0], [0, HPC], [1, 128]])
                        sin4 = _ap(sin_t, 0,
                                   [ca[0], [0, HPC], [64, 2], [1, 64]])
                        tmp = a_t.tile([128, HPC, D], bf16, tag='rtmp')
                        eng.tensor_tensor(
                            out=tmp[:].rearrange('p h (u d) -> p h u d', u=2),
                            in0=swap, in1=sin4, op=OP.mult)
                        eng.tensor_tensor(out=out[:], in0=src3, in1=cos4,
                                          op=OP.mult)
                        eng.tensor_add(out=out[:], in0=out[:], in1=tmp[:])

                    def blk4(ap20, reps=32):
                        # (128,20) -> (128,5,4,reps) block broadcast
                        a = list(ap20.ap)
                        st = a[-1][0]
                        return bass.AP(tensor=ap20.tensor, offset=ap20.offset,
                                       ap=[a[0], [4 * st, HPC], [st, 4],
                                           [0, reps]])

                    def hb(ap5, reps=4):
                        # (128,5) -> (128,5,reps) broadcast
                        a = list(ap5.ap)
                        return bass.AP(tensor=ap5.tensor, offset=ap5.offset,
                                       ap=[a[0], [a[-1][0], HPC], [0, reps]])

                    def v4(x):
                        return x.rearrange('p h (b e) -> p h b e', e=32)

                    def emit_tail(t):
                        qf, kf, vf = stash.pop(t)
                        # rms of pre-rope q/k (rope is norm-preserving)
                        msq = a_t.tile([128, 2, HPC], f32, tag='msq')
                        for h in range(HPC):
                            nc.scalar.activation(
                                out=scr_sq[:, 0:D],
                                in_=qf[:, h * D:(h + 1) * D],
                                func=AF.Square, bias=zero_sb[:],
                                accum_out=msq[:, 0, h:h + 1])
                        ksq = a_t.tile([128, HPC, D], bf16, tag='ksq')
                        kf3 = kf[:].rearrange('p (h d) -> p h d', h=HPC)
                        nc.vector.tensor_tensor(out=ksq[:], in0=kf3,
                                                in1=kf3, op=OP.mult)
                        nc.vector.tensor_reduce(out=msq[:, 1, :],
                                                in_=ksq[:], axis=AX.X,
                                                op=OP.add)
                        _rsqrt_vec(nc, a_t, msq[:], msq[:],
                                   float(1.0 / D), EPS, 'rqk')
                        # rope (q on vector, k on gpsimd)
                        zq = a_t.tile([128, HPC, D], bf16, tag='zq')
                        rope(nc.vector, qf, cq_sb[:, t, :], sq_sb[:, t, :],
                             zq)
                        zk = a_t.tile([128, HPC, D], bf16, tag='zk')
                        rope(nc.gpsimd, kf, ck_sb[:, t, :], sk_sb[:, t, :],
                             zk)
                        # block amax; amn = amax*rstd (q,k) or amax (v)
                        amn = a_t.tile([128, 3, NBLK], f32, tag='amn')
                        nc.vector.tensor_reduce(
                            out=amn[:, 0, :], in_=v4(zq[:]), axis=AX.X,
                            op=OP.max, apply_absolute_value=True)
                        nc.vector.tensor_reduce(
                            out=amn[:, 1, :], in_=v4(zk[:]), axis=AX.X,
                            op=OP.max, apply_absolute_value=True)
                        nc.vector.tensor_reduce(
                            out=amn[:, 2, :],
                            in_=vf[:].rearrange('p (h b e) -> p h b e',
                                                h=HPC, e=32),
                            axis=AX.X, op=OP.max, apply_absolute_value=True)
                        for i in range(2):
                            nc.vector.tensor_tensor(
                                out=amn[:, i, :].rearrange(
                                    'p (h b) -> p h b', h=HPC),
                                in0=amn[:, i, :].rearrange(
                                    'p (h b) -> p h b', h=HPC),
                                in1=hb(msq[:, i, :]), op=OP.mult)
                        nc.vector.tensor_scalar_max(out=amn[:], in0=amn[:],
                                                    scalar1=1e-12)
                        eb = a_t.tile([128, 3, NBLK], i32, tag='eb')
                        nc.vector.tensor_single_scalar(
                            out=eb[:], in_=amn[:].bitcast(i32), scalar=23,
                            op=OP.logical_shift_right)
                        sc = a_t.tile([128, 3, NBLK], f32, tag='sc')
                        nc.vector.tensor_scalar(
                            out=sc[:].bitcast(i32), in0=eb[:], scalar1=-1,
                            scalar2=260, op0=OP.mult, op1=OP.add)
                        nc.vector.tensor_single_scalar(
                            out=sc[:].bitcast(i32), in_=sc[:].bitcast(i32),
                            scalar=23, op=OP.logical_shift_left)
                        isc = a_t.tile([128, 3, NBLK], f32, tag='isc')
                        nc.vector.tensor_single_scalar(
                            out=isc[:].bitcast(i32), in_=eb[:], scalar=6,
                            op=OP.subtract)
                        nc.vector.tensor_single_scalar(
                            out=isc[:].bitcast(i32), in_=isc[:].bitcast(i32),
                            scalar=23, op=OP.logical_shift_left)
                        msc = a_t.tile([128, 2, NBLK], f32, tag='msc')
                        for i in range(2):
                            nc.vector.tensor_tensor(
                                out=msc[:, i, :].rearrange(
                                    'p (h b) -> p h b', h=HPC),
                                in0=sc[:, i, :].rearrange(
                                    'p (h b) -> p h b', h=HPC),
                                in1=hb(msq[:, i, :]), op=OP.mult)
                        # quantize q (vector)
                        ys = a_t.tile([128, HPC, D], bf16, tag='ys')
                        q8 = a_t.tile([128, HPC, D], fp8, tag='q8')
                        qd = a_t.tile([128, HPC, D], bf16, tag='qd')
                        nc.vector.tensor_tensor(out=v4(ys[:]), in0=v4(zq[:]),
                                                in1=blk4(msc[:, 0, :]),
                                                op=OP.mult)
                        nc.vector.tensor_scalar(out=q8[:], in0=ys[:],
                                                scalar1=-112.0,
                                                scalar2=112.0,
                                                op0=OP.max, op1=OP.min)
                        nc.vector.tensor_tensor(out=v4(qd[:]), in0=v4(q8[:]),
                                                in1=blk4(isc[:, 0, :]),
                                                op=OP.mult)
                        # quantize k (gpsimd mults, vector fp8 cast)
                        ysk = a_t.tile([128, HPC, D], bf16, tag='ys')
                        k8 = a_t.tile([128, HPC, D], fp8, tag='q8')
                        kd = a_t.tile([128, HPC, D], bf16, tag='kd')
                        nc.gpsimd.tensor_tensor(out=v4(ysk[:]),
                                                in0=v4(zk[:]),
                                                in1=blk4(msc[:, 1, :]),
                                                op=OP.mult)
                        nc.vector.tensor_scalar(out=k8[:], in0=ysk[:],
                                                scalar1=-112.0,
                                                scalar2=112.0,
                                                op0=OP.max, op1=OP.min)
                        nc.gpsimd.tensor_tensor(out=v4(kd[:]), in0=v4(k8[:]),
                                                in1=blk4(isc[:, 1, :]),
                                                op=OP.mult)
                        # quantize v (vector; deq straight into vd_sb)
                        ysv = a_t.tile([128, HPC, D], bf16, tag='ys')
                        v8 = a_t.tile([128, HPC, D], fp8, tag='q8')
                        nc.vector.tensor_tensor(
                            out=v4(ysv[:]),
                            in0=v4(vf[:].rearrange('p (h d) -> p h d',
                                                   h=HPC)),
                            in1=blk4(sc[:, 2, :]), op=OP.mult)
                        nc.vector.tensor_scalar(out=v8[:], in0=ysv[:],
                                                scalar1=-112.0,
                                                scalar2=112.0,
                                                op0=OP.max, op1=OP.min)
                        nc.vector.tensor_tensor(
                            out=v4(vd_sb[:, t, :, 0:D]), in0=v4(v8[:]),
                            in1=blk4(isc[:, 2, :]), op=OP.mult)
                        # transpose qd/kd into qT/kT
                        for src, dstT in ((qd, qT), (kd, kT)):
                            tp = psT.tile([128, 640], bf16, tag='tp')
                            for h in range(HPC):
                                nc.tensor.transpose(
                                    tp[:, h * 128:(h + 1) * 128],
                                    src[:, h, :], ident_b[:])
                            nc.vector.tensor_copy(
                                out=dstT[:, :, t * 128:(t + 1) * 128],
                                in_=tp[:].rearrange('p (h d) -> p h d',
                                                    h=HPC))

                    def emit_attn_h(qb, h):
                        nkt = 4 * qb + 4
                        if True:
                            dps = psD.tile([128, 512], f32, tag='dps')
                            ops = ops_ps.tile([128, 512], f32, tag='ops')
                            for kt in range(nkt):
                                sp = ps512.tile([128, 512], f32, tag='mm')
                                o = kt - 4 * qb
                                nc.tensor.matmul(
                                    sp[:],
                                    kT[:, h, kt * 128:(kt + 1) * 128],
                                    qT[:, h, qb * 512:(qb + 1) * 512],
                                    start=True, stop=True)
                                pT = pT_pool.tile([128, 512], bf16, tag='pT')
                                nc.scalar.activation(out=pT[:], in_=sp[:],
                                                     func=AF.Exp,
                                                     bias=zero_sb[:],
                                                     scale=INV_SQRT_D)
                                if o >= 0:
                                    nc.gpsimd.affine_select(
                                        out=pT[:], in_=pT[:],
                                        compare_op=OP.is_ge, fill=0.0,
                                        base=-128 * o, pattern=[[1, 512]],
                                        channel_multiplier=-1)
                                nc.tensor.matmul(dps[:], ones128[:], pT[:],
                                                 start=(kt == 0),
                                                 stop=(kt == nkt - 1))
                                nc.tensor.matmul(ops[:],
                                                 vd_sb[:, kt, h, 0:128],
                                                 pT[:],
                                                 start=(kt == 0),
                                                 stop=(kt == nkt - 1))
                            rd = b_t.tile([128, 512], f32, tag='rd')
                            nc.vector.reciprocal_approx_fast(out=rd[:],
                                                             in_=dps[:])
                            nc.vector.tensor_tensor(
                                out=attnT[:, h, qb * 512:(qb + 1) * 512],
                                in0=ops[:], in1=rd[:], op=OP.mult)

                    wo_sb = a_w.tile([128, HPC, C], bf16)
                    nc.sync.dma_start(
                        out=wo_sb[:],
                        in_=wout_d.ap().rearrange('(h p) c -> p h c', p=128))
                    # rs_in row layout: [t0:512 | t1024:1536 | t512:1024
                    # | t1536:2048] so each RS half is contiguous.
                    rowblk = {tt: i for i, tt in enumerate(
                        (0, 1, 2, 3, 8, 9, 10, 11, 4, 5, 6, 7,
                         12, 13, 14, 15))}
                    grp = [[2 * i, 2 * i + 1] for i in range(n_cores // 2)]

                    def oproj(tt):
                        ob = b_t.tile([128, C], bf16, tag='ob', name='ob')
                        for ci, (lo, hi) in enumerate(((0, 512),
                                                       (512, 1024),
                                                       (1024, C))):
                            ps = ps512.tile([128, 512], f32, tag='mm',
                                            name='oprojps')
                            for h in range(HPC):
                                nc.tensor.matmul(
                                    ps[:, 0:hi - lo],
                                    attnT[:, h, tt * 128:(tt + 1) * 128],
                                    wo_sb[:, h, lo:hi],
                                    start=(h == 0), stop=(h == HPC - 1))
                            if ci == 2:
                                nc.scalar.copy(out=ob[:, lo:hi],
                                               in_=ps[:, 0:hi - lo])
                            else:
                                nc.vector.tensor_copy(
                                    out=ob[:, lo:hi], in_=ps[:, 0:hi - lo])
                        r = rowblk[tt]
                        nc.sync.dma_start(
                            out=rs_in[r * 128:(r + 1) * 128, :], in_=ob[:])

                    # ---- interleaved A+B+C emission ----
                    for t in range(NT):
                        emit_head(t)
                        if t >= 1:
                            emit_tail(t - 1)
                        if t >= 4 and t % 4 == 0:
                            qb = t // 4 - 1
                            for h in range(HPC):
                                emit_attn_h(qb, h)
                            for tt in range(4 * qb, 4 * qb + 4):
                                oproj(tt)
                            if qb == 2:
                                nc.gpsimd.collective_compute(
                                    'ReduceScatter', OP.add,
                                    ins=[rs_in[0:1024, :].opt()],
                                    outs=[rs_out[0:512, :].opt()],
                                    replica_groups=grp)
                    emit_tail(NT - 1)
                    for h in range(HPC):
                        emit_attn_h(QB - 1, h)
                    for tt in range(12, 16):
                        oproj(tt)
                    nc.gpsimd.collective_compute(
                        'ReduceScatter', OP.add,
                        ins=[rs_in[1024:2048, :].opt()],
                        outs=[rs_out[512:1024, :].opt()],
                        replica_groups=grp)

            # ====== phase D: residual + MLP over my T/2 tokens ======
            with contextlib.ExitStack() as pd:
                d_t = pd.enter_context(tc.tile_pool(name='d_t', bufs=2))
                d_big = pd.enter_context(tc.tile_pool(name='d_big', bufs=1))
                x2_sb = d_big.tile([128, NH, C], f32)
                xn2T = d_big.tile([128, 10, T // 2], bf16)
                h2T = d_big.tile([128, F // 128, T // 2], bf16)
                rinv_sb = d_big.tile([128, NH], f32)

                with tc.tile_pool(name='d_ps', bufs=4, space='PSUM') as d_ps, \
                     tc.tile_pool(name='dt_ps', bufs=2,
                                  space='PSUM') as dt_ps:
                    for tt in range(NH):
                        rsx = d_t.tile([128, C], bf16, tag='rsx')
                        nc.sync.dma_start(
                            out=rsx[:],
                            in_=rs_out[tt * 128:(tt + 1) * 128, :])
                        xht = d_t.tile([128, C], f32, tag='xht')
                        nc.sync.dma_start(
                            out=xht[:],
                            in_=xh_d[tt * 128:(tt + 1) * 128, :])
                        nc.vector.tensor_add(out=x2_sb[:, tt, :],
                                             in0=rsx[:], in1=xht[:])
                        ssq2 = d_t.tile([128, 1], f32, tag='ssq2')
                        nc.scalar.activation(out=scr_sq[:],
                                             in_=x2_sb[:, tt, :],
                                             func=AF.Square, bias=zero_sb[:],
                                             accum_out=ssq2[:])
                        m2 = d_t.tile([128, 1], f32, tag='m2')
                        nc.vector.tensor_scalar(out=m2[:], in0=ssq2[:],
                                                scalar1=float(1.0 / C),
                                                scalar2=EPS,
                                                op0=OP.mult, op1=OP.add)
                        nc.vector.reciprocal_approx_fast(
                            out=rinv_sb[:, tt:tt + 1], in_=m2[:])
                        for jg, (lo, hi) in enumerate(((0, 4), (4, 8),
                                                      (8, 10))):
                            tp2 = dt_ps.tile([128, 512], f32, tag='tp2')
                            for j in range(lo, hi):
                                nc.tensor.transpose(
                                    tp2[:, (j - lo) * 128:(j - lo + 1) * 128],
                                    x2_sb[:, tt, j * 128:(j + 1) * 128],
                                    ident_f[:])
                            nc.vector.tensor_copy(
                                out=xn2T[:, lo:hi, tt * 128:(tt + 1) * 128],
                                in_=tp2[:, 0:(hi - lo) * 128].rearrange(
                                    'p (j d) -> p j d', d=128))

                    # fc1: j-loop with LDW amortized over two 512 chunks
                    with tc.tile_pool(name='wf1', bufs=5) as wf1_pool:
                        for fi in range(F // 128):
                            wf1 = wf1_pool.tile([128, 10, 128], bf16,
                                                tag='wf1')
                            nc.sync.dma_start(
                                out=wf1[:],
                                in_=wfc1_d[:, fi * 128:(fi + 1) * 128]
                                .rearrange('(j p) c -> p j c', p=128))
                            hp0 = d_ps.tile([128, 512], f32, tag='hps')
                            hp1 = d_ps.tile([128, 512], f32, tag='hps')
                            for j in range(10):
                                nc.tensor.matmul(hp0[:], wf1[:, j, :],
                                                 xn2T[:, j, 0:512],
                                                 start=(j == 0),
                                                 stop=(j == 9))
                                nc.tensor.matmul(hp1[:], wf1[:, j, :],
                                                 xn2T[:, j, 512:1024],
                                                 start=(j == 0),
                                                 stop=(j == 9))
                            for ci, hp in ((0, hp0), (1, hp1)):
                                hrelu = d_t.tile([128, 512], bf16,
                                                 tag='hrelu')
                                nc.scalar.activation(out=hrelu[:], in_=hp[:],
                                                     func=AF.Relu,
                                                     bias=zero_sb[:])
                                nc.vector.tensor_mul(
                                    out=h2T[:, fi,
                                            ci * 512:(ci + 1) * 512],
                                    in0=hrelu[:], in1=hrelu[:])

                # fc2: c-halves x tt-quads; 2 matmuls (640 cols) per lhsT
                with tc.tile_pool(name='y_ps', bufs=4, space='PSUM') as y_ps, \
                     tc.tile_pool(name='wf2', bufs=3) as wf2_pool:
                    for clo, chi in ((0, 640), (640, C)):
                        for ttg in range(2):
                            yps = [y_ps.tile([128, 640], f32, tag='yps',
                                             name='yps')
                                   for _ in range(4)]
                            for f2 in range(F // 256):
                                wf2 = wf2_pool.tile([128, 2, 640], bf16,
                                                    tag='wf2')
                                nc.sync.dma_start(
                                    out=wf2[:],
                                    in_=wfc2_d[f2 * 256:(f2 + 1) * 256,
                                               clo:chi]
                                    .rearrange('(u p) c -> p u c', p=128))
                                for u in range(2):
                                    fi = 2 * f2 + u
                                    st = (fi == 0)
                                    sp_ = (fi == F // 128 - 1)
                                    for i in range(4):
                                        tt = 4 * ttg + i
                                        lhsT = h2T[:, fi,
                                                   tt * 128:(tt + 1) * 128]
                                        nc.tensor.matmul(
                                            yps[i][:, 0:512], lhsT,
                                            wf2[:, u, 0:512],
                                            start=st, stop=sp_)
                                        nc.tensor.matmul(
                                            yps[i][:, 512:640], lhsT,
                                            wf2[:, u, 512:640],
                                            start=st, stop=sp_)
                            for i in range(4):
                                tt = 4 * ttg + i
                                yo = d_t.tile([128, 640], f32, tag='yo')
                                nc.vector.scalar_tensor_tensor(
                                    out=yo[:], in0=yps[i][:],
                                    scalar=rinv_sb[:, tt:tt + 1],
                                    in1=x2_sb[:, tt, clo:chi],
                                    op0=OP.mult, op1=OP.add)
                                nc.sync.dma_start(
                                    out=y_d[tt * 128:(tt + 1) * 128,
                                            clo:chi],
                                    in_=yo[:])

    nc.compile()
    return nc


_CACHE = {}


def _get_nc(t_len=T):
    if t_len not in _CACHE:
        _CACHE[t_len] = build_nc(t_len)
    return _CACHE[t_len]


def make_in_maps(x, rotary_pos_emb, ln1_w, w_qkv, qn_w, kn_w, w_out, ln2_w,
                 w_fc1, w_fc2, t_len=T):
    """Host-side sharding prep. Returns list of per-core input dicts."""
    x = np.asarray(x, np.float32)
    rot = np.asarray(rotary_pos_emb, np.float32)
    cos = np.cos(rot).astype(np.float32)
    sin = np.sin(rot).astype(np.float32)
    sinneg = np.concatenate([-sin[:, :64], sin[:, :64]], axis=-1)
    qn = np.asarray(qn_w, np.float32)
    kn = np.asarray(kn_w, np.float32)
    cosq = (cos * qn).astype(ml_dtypes.bfloat16)
    sinq = (sinneg * qn).astype(ml_dtypes.bfloat16)
    cosk = (cos * kn).astype(ml_dtypes.bfloat16)
    sink = (sinneg * kn).astype(ml_dtypes.bfloat16)
    w_qkv_f = (np.asarray(w_qkv, np.float32)
               * np.asarray(ln1_w, np.float32)[:, None]).reshape(C, 3, H, D)
    w_fc1_f = (np.asarray(w_fc1, np.float32)
               * np.asarray(ln2_w, np.float32)[:, None]
               ).astype(ml_dtypes.bfloat16)
    w_fc2_b = np.asarray(w_fc2, np.float32).astype(ml_dtypes.bfloat16)
    wo = np.asarray(w_out, np.float32).reshape(H, D, C)

    in_maps = []
    for c in range(N_CORES):
        b, hg = c // 2, c % 2
        heads = slice(hg * HPC, (hg + 1) * HPC)
        wq = np.ascontiguousarray(
            w_qkv_f[:, :, heads, :].reshape(C, 3 * CPC)
        ).astype(ml_dtypes.bfloat16)
        w_outp = np.ascontiguousarray(
            wo[heads].reshape(CPC, C)).astype(ml_dtypes.bfloat16)
        in_maps.append({
            'x': np.ascontiguousarray(x[b]),
            'xh': np.ascontiguousarray(x[b, hg * T // 2:(hg + 1) * T // 2]),
            'w_qkv': wq,
            'cosq': cosq, 'sinq': sinq, 'cosk': cosk, 'sink': sink,
            'w_out': w_outp,
            'w_fc1': np.ascontiguousarray(w_fc1_f),
            'w_fc2': np.ascontiguousarray(w_fc2_b),
        })
    return in_maps


def assemble_output(results, t_len=T):
    out = np.zeros((B, t_len, C), np.float32)
    for c in range(N_CORES):
        b, hg = c // 2, c % 2
        out[b, hg * t_len // 2:(hg + 1) * t_len // 2] = results[c]['y']
    return out


def kernel(**inputs):
    nc = _get_nc(T)
    in_maps = make_in_maps(**inputs)
    res = bass_utils.run_bass_kernel_spmd(nc, in_maps,
                                          core_ids=list(range(N_CORES)))
    return assemble_output(res.results)
